# revision 17
# baseline (speedup 1.0000x reference)
"""GAT attention-locator message passing on 8 Trainium2 NeuronCores.

Strategy: shard by destination node (N/8 dst nodes per core) so the
segment softmax is fully core-local.

Key factorization: with the global softmax shift C (valid per segment by
shift invariance) and the continuity of leaky_relu at 0, each edge weight
is  w = exp(as+ad-C)            if as+ad > 0
     w = exp(0.2(as+ad)-C)      otherwise
Both branches factor into (src term)x(dst term):
     w = A1[src]*B1[dst]  or  A2[src]*B2[dst]
  A1 = exp(as-C/2), B1 = exp(ad-C/2), A2 = exp(0.2as-C/2), B2 = exp(0.2ad-C/2).
The host classifies edges by sign(as+ad) (computed on CPU for LAYOUT only;
misclassification at the boundary is harmless since both branches agree
at 0).  The device then needs NO per-edge scalars at all:
 - A-terms are folded into pre-scaled gather tables (built on device):
     xc row m (bf16, 384 slots): [xp*A1 (128) | A1 | xp*A2 (128) | A2 | pad]
 - B-terms are applied per NODE after PSUM accumulation.
Per edge: one 768B dma_gather row + a shared 0/1 mask + one bf16 matmul
accumulating messages and the denominator (A column) into PSUM.

Per chunk (125 dst nodes): 4 gathers (sign x int16-index-range split),
one tiny per-node gather for ad (B-terms), T matmuls, per-node combine:
  out = (B1*psum1 + B2*psum2) / (B1*den1 + B2*den2 + eps) + x_proj + bias
"""

import math
import sys
import types

import numpy as np

# ------------------------------------------------------------ configuration
def make_cfg(N, E, F, HC, NCORES, CN, SPLIT, CSHIFT=12.0):
    NPC = N // NCORES
    assert NPC % CN == 0
    NPAD = ((N + 1023) // 1024) * 1024  # divisible by 8*128
    OWNK = (NPC + 127) // 128  # rows per partition in core-local tables
    cfg = dict(
        N=N, E=E, F=F, HC=HC, NCORES=NCORES, NPC=NPC, CN=CN,
        CHUNKS=NPC // CN, GCHUNKS=NCORES * (NPC // CN), SPLIT=SPLIT,
        NPAD=NPAD, NTILES=NPAD // 128, NGRP=NPAD // 1024,
        OWNK=OWNK, NOWN=OWNK * 128,
        CSHIFT=CSHIFT, EPS=1e-16, NEG=0.2,
    )
    assert cfg["NOWN"] <= NPAD
    return cfg


FULL_CFG = make_cfg(N=50000, E=1600000, F=256, HC=128, NCORES=8, CN=125,
                    SPLIT=32768)

_PROG_CACHE: dict = {}


def _ensure_ntff_hook():
    try:
        import antenv.axon_hooks  # noqa: F401

        return
    except ImportError:
        pass
    try:
        from trn_agent_boot.trn_boot import _ntff_profile_via_ctypes

        hook = _ntff_profile_via_ctypes("/opt/axon/libaxon_pjrt.so")
    except Exception:
        hook = None
    mod = types.ModuleType("antenv.axon_hooks")
    mod.get_axon_ntff_profile_hook = lambda: hook
    mod.set_axon_ntff_profile_hook = lambda h: None
    sys.modules["antenv.axon_hooks"] = mod


# ---------------------------------------------------------------- host prep
def _prep_edges(cfg, src, dst, possign):
    """4-way (sign, src-range) classed edge layout per dst chunk."""
    G, CN, SPLIT = cfg["GCHUNKS"], cfg["CN"], cfg["SPLIT"]
    src = src.astype(np.int64)
    dst = dst.astype(np.int64)
    g = dst // CN
    nl = dst % CN
    lowm = src < SPLIT
    # class: 0=pos-lo 1=pos-hi 2=neg-lo 3=neg-hi
    cls = np.where(possign, 0, 2) + np.where(lowm, 0, 1)

    key = g * 4 + cls
    order = np.argsort(key, kind="stable")
    g_s, nl_s, src_s, cls_s, key_s = (
        g[order], nl[order], src[order], cls[order], key[order]
    )
    starts = np.searchsorted(key_s, np.arange(4 * G))
    counts = np.bincount(key_s, minlength=4 * G)
    TB = [max(1, math.ceil(counts[c::4].max() / 128)) for c in range(4)]
    T = sum(TB)
    off = np.cumsum([0] + TB[:-1]) * 128  # class position offsets

    rank = np.arange(len(order)) - starts[key_s]
    pos = off[cls_s] + rank
    p = (pos % 128).astype(np.int64)
    t = (pos // 128).astype(np.int64)

    dstl = np.full((G, 128, T), -1.0, np.float32)
    dstl[g_s, p, t] = nl_s.astype(np.float32)

    idx = []
    for c in range(4):
        m = cls_s == c
        a = np.zeros((G, TB[c] * 128), np.int16)
        v = src_s[m] - (SPLIT if c % 2 else 0)
        a[g_s[m], rank[m]] = v.astype(np.int16)
        idx.append(a)

    def wrap(a):
        tb8 = a.shape[1] // 16
        w = a.reshape(G, tb8, 16).transpose(0, 2, 1)
        return np.ascontiguousarray(np.tile(w, (1, 8, 1)))

    return dict(TB=tuple(TB), T=T, dstl=dstl,
                idx=[wrap(a) for a in idx])


def host_prep(cfg, x, edge_index, v_mapping, W_src, att_src, att_dst, bias):
    N, F = cfg["N"], cfg["F"]
    NCORES, NPC, CN, CHUNKS = cfg["NCORES"], cfg["NPC"], cfg["CN"], cfg["CHUNKS"]
    NPAD, OWNK = cfg["NPAD"], cfg["OWNK"]

    def _fold(att):
        a = att[0]
        a = a / max(np.linalg.norm(a), 1e-12)
        return v_mapping.T @ a

    u_src = _fold(att_src)
    u_dst = _fold(att_dst)
    w_ext = np.concatenate(
        [W_src, u_src[:, None], u_dst[:, None]], axis=1
    ).astype(np.float32)

    x_pad = np.zeros((NPAD, F), np.float32)
    x_pad[:N] = x

    # sign classification (layout only; boundary-safe because the two
    # leaky-relu branches agree at 0)
    a_src_h = x @ u_src.astype(np.float32)
    a_dst_h = x @ u_dst.astype(np.float32)
    src, dst = edge_index[0].astype(np.int64), edge_index[1].astype(np.int64)
    possign = (a_src_h[src] + a_dst_h[dst]) > 0

    tabs = _prep_edges(cfg, src, dst, possign)

    import ml_dtypes
    iota_n = np.tile(np.arange(CN, dtype=np.float32), (128, 1))
    bias_b = np.tile(bias[None, :], (128, 1)).astype(np.float32)
    ident = np.eye(128, dtype=np.float32)

    # per-chunk per-node small-gather indices (core-independent)
    nidx = np.zeros((CHUNKS, 128), np.int16)
    for c in range(CHUNKS):
        nidx[c] = c * CN + np.minimum(np.arange(128), CN - 1)
    nidx_w = np.ascontiguousarray(
        np.tile(nidx.reshape(CHUNKS, 8, 16).transpose(0, 2, 1), (1, 8, 1))
    )  # [CHUNKS, 128, 8]

    T = tabs["T"]
    iota_rep = np.ascontiguousarray(
        np.broadcast_to(np.arange(CN, dtype=np.float32), (128, T, CN))
    ).astype(ml_dtypes.bfloat16)
    p128 = np.arange(128, dtype=np.int32)
    in_maps = []
    for k in range(NCORES):
        sl = slice(k * CHUNKS, (k + 1) * CHUNKS)
        base = k * NPC
        in_maps.append(
            {
                "x_pad": x_pad,
                "w_ext": w_ext,
                "iota_n": iota_n,
                "bias_b": bias_b,
                "ident": ident,
                "own_off": (base + p128 * OWNK).astype(np.int32)[:, None],
                "gidx0": tabs["idx"][0][sl],
                "gidx1": tabs["idx"][1][sl],
                "gidx2": tabs["idx"][2][sl],
                "gidx3": tabs["idx"][3][sl],
                "nidx": nidx_w,
                "iota_rep": iota_rep,
                "dstl": tabs["dstl"][sl].astype(ml_dtypes.bfloat16),
            }
        )
    return in_maps, tabs["TB"]


# ------------------------------------------------------------- bass program
def build_program(cfg, TB):
    import concourse.mybir as mybir
    import concourse.tile as tile
    from concourse import bacc
    from concourse.bass import IndirectOffsetOnAxis

    F, HC = cfg["F"], cfg["HC"]
    NCORES, CN, CHUNKS = cfg["NCORES"], cfg["CN"], cfg["CHUNKS"]
    NPC, NPAD, NTILES, NGRP = cfg["NPC"], cfg["NPAD"], cfg["NTILES"], cfg["NGRP"]
    SPLIT, OWNK, NOWN = cfg["SPLIT"], cfg["OWNK"], cfg["NOWN"]
    CSHIFT, EPS, NEG = cfg["CSHIFT"], cfg["EPS"], cfg["NEG"]
    KB = F // 128
    XW = 3 * HC  # combined row slots: [xpA1(128)|A1|xpA2(128)|A2|pad] in 384
    C2 = CSHIFT / 2.0

    T = sum(TB)
    TPOS = TB[0] + TB[1]
    f32 = mybir.dt.float32
    bf16 = mybir.dt.bfloat16
    i16 = mybir.dt.int16
    Alu = mybir.AluOpType
    Act = mybir.ActivationFunctionType

    nc = bacc.Bacc("TRN2", target_bir_lowering=False, debug=False, num_devices=NCORES)

    x_h = nc.dram_tensor("x_pad", [NPAD, F], f32, kind="ExternalInput")
    w_h = nc.dram_tensor("w_ext", [F, HC + 2], f32, kind="ExternalInput")
    iota_h = nc.dram_tensor("iota_n", [128, CN], f32, kind="ExternalInput")
    biasb_h = nc.dram_tensor("bias_b", [128, HC], f32, kind="ExternalInput")
    ident_h = nc.dram_tensor("ident", [128, 128], f32, kind="ExternalInput")
    ownoff_h = nc.dram_tensor("own_off", [128, 1], mybir.dt.int32, kind="ExternalInput")
    gidx_h = [
        nc.dram_tensor(f"gidx{c}", [CHUNKS, 128, TB[c] * 8], i16,
                       kind="ExternalInput")
        for c in range(4)
    ]
    nidx_h = nc.dram_tensor("nidx", [CHUNKS, 128, 8], i16, kind="ExternalInput")
    iotar_h = nc.dram_tensor("iota_rep", [128, T, CN], bf16, kind="ExternalInput")
    dstl_h = nc.dram_tensor("dstl", [CHUNKS, 128, T], bf16, kind="ExternalInput")
    out_h = nc.dram_tensor("out", [NPC, HC], f32, kind="ExternalOutput")

    with tile.TileContext(nc) as tc:
        with tc.tile_pool(name="dram", bufs=1, space="DRAM") as dpool:
            xc_d = dpool.tile([NPAD, XW], bf16)     # combined scaled rows
            xr_d = dpool.tile([NPAD, HC], bf16)     # raw x_proj rows
            scT_d = dpool.tile([NPAD, 1], f32)      # a_dst node-contiguous
            sc3_d = dpool.tile([NOWN, HC], bf16)    # [ad_hi|ad_lo|junk] rows
            ownxp_d = dpool.tile([NOWN, HC], bf16)  # own raw x_proj rows

            # ---------------- Phase P: projection + tables ----------------
            with (
                tc.tile_pool(name="pconst", bufs=1) as cpool,
                tc.tile_pool(name="pio", bufs=3) as iopool,
                tc.tile_pool(name="pwork", bufs=4) as wpool,
                tc.tile_pool(name="ppsum", bufs=2, space="PSUM") as pp,
            ):
                wk = []
                for kb in range(KB):
                    wt = cpool.tile([128, HC + 2], f32, tag=f"w{kb}")
                    nc.sync.dma_start(
                        out=wt[:], in_=w_h.ap()[kb * 128 : (kb + 1) * 128, :]
                    )
                    wk.append(wt)
                ident = cpool.tile([128, 128], f32)
                nc.sync.dma_start(out=ident[:], in_=ident_h.ap())
                nC2p = cpool.tile([128, 1], f32)
                nc.vector.memset(nC2p[:], -C2)
                adst_sbuf = cpool.tile([128, NTILES], f32)

                x_r = x_h.ap().rearrange("(g a p) f -> g p a f", a=8, p=128)
                xc_r = xc_d[:].rearrange("(g a p) c -> g p a c", a=8, p=128)
                xr_r = xr_d[:].rearrange("(g a p) c -> g p a c", a=8, p=128)
                for gi in range(NGRP):
                    xt = iopool.tile([128, 8, F], f32)
                    nc.sync.dma_start(out=xt[:], in_=x_r[gi])
                    xec = iopool.tile([128, 8, XW], bf16, tag="xec")
                    nc.gpsimd.memset(xec[:, :, 2 * HC + 2 : XW], 0.0)
                    xer = iopool.tile([128, 8, HC], bf16, tag="xer")
                    for a in range(8):
                        nt = gi * 8 + a
                        px = pp.tile([128, HC + 2], f32, tag="px")
                        for kb in range(KB):
                            pt = pp.tile([128, 128], f32, tag=f"pt{kb}")
                            nc.tensor.transpose(
                                out=pt[:],
                                in_=xt[:, a, kb * 128 : (kb + 1) * 128],
                                identity=ident[:],
                            )
                            xT = wpool.tile([128, 128], f32, tag=f"xT{kb}")
                            if kb == 0:
                                nc.vector.tensor_copy(out=xT[:], in_=pt[:])
                            else:
                                nc.scalar.copy(out=xT[:], in_=pt[:])
                            nc.tensor.matmul(
                                px[:],
                                lhsT=xT[:],
                                rhs=wk[kb][:],
                                start=(kb == 0),
                                stop=(kb == KB - 1),
                            )
                        A1f = wpool.tile([128, 1], f32, tag="A1f")
                        nc.scalar.activation(
                            out=A1f[:], in_=px[:, HC : HC + 1], func=Act.Exp,
                            bias=nC2p[:], scale=1.0,
                        )
                        A2f = wpool.tile([128, 1], f32, tag="A2f")
                        nc.scalar.activation(
                            out=A2f[:], in_=px[:, HC : HC + 1], func=Act.Exp,
                            bias=nC2p[:], scale=NEG,
                        )
                        nc.vector.tensor_scalar(
                            out=xec[:, a, 0:HC], in0=px[:, 0:HC], scalar1=A1f[:],
                            scalar2=None, op0=Alu.mult,
                        )
                        nc.gpsimd.tensor_copy(
                            out=xec[:, a, HC : HC + 1], in_=A1f[:]
                        )
                        nc.vector.tensor_scalar(
                            out=xec[:, a, HC + 1 : 2 * HC + 1], in0=px[:, 0:HC],
                            scalar1=A2f[:], scalar2=None, op0=Alu.mult,
                        )
                        nc.gpsimd.tensor_copy(
                            out=xec[:, a, 2 * HC + 1 : 2 * HC + 2], in_=A2f[:]
                        )
                        nc.vector.tensor_copy(out=xer[:, a, :], in_=px[:, 0:HC])
                        nc.vector.tensor_copy(
                            out=adst_sbuf[:, nt : nt + 1],
                            in_=px[:, HC + 1 : HC + 2],
                        )
                    nc.sync.dma_start(out=xc_r[gi], in_=xec[:])
                    nc.sync.dma_start(out=xr_r[gi], in_=xer[:])
                scT_r = scT_d[:].rearrange("(nt p) one -> p (nt one)", p=128)
                nc.sync.dma_start(out=scT_r, in_=adst_sbuf[:])

                ownoff = cpool.tile([128, 1], mybir.dt.int32)
                nc.sync.dma_start(out=ownoff[:], in_=ownoff_h.ap())
                own_ad = cpool.tile([128, OWNK], f32)
                nc.gpsimd.indirect_dma_start(
                    out=own_ad[:], out_offset=None, in_=scT_d[:],
                    in_offset=IndirectOffsetOnAxis(ap=ownoff[:], axis=0),
                )
                sc3s = cpool.tile([128, OWNK, HC], bf16)
                nc.gpsimd.memset(sc3s[:], 0.0)
                nc.vector.tensor_copy(out=sc3s[:, :, 0], in_=own_ad[:])
                nc.vector.tensor_tensor(
                    out=sc3s[:, :, 1], in0=own_ad[:], in1=sc3s[:, :, 0],
                    op=Alu.subtract,
                )
                sc3_r = sc3_d[:].rearrange("(p i) c -> p i c", p=128)
                nc.sync.dma_start(out=sc3_r, in_=sc3s[:])
                own_xe = cpool.tile([128, OWNK * HC], bf16)
                nc.gpsimd.indirect_dma_start(
                    out=own_xe[:], out_offset=None, in_=xr_d[:],
                    in_offset=IndirectOffsetOnAxis(ap=ownoff[:], axis=0),
                )
                oxp_r = ownxp_d[:].rearrange("(p i) c -> p (i c)", p=128)
                nc.sync.dma_start(out=oxp_r, in_=own_xe[:])

            # ---------------- Phase E: edges ------------------------------
            with (
                tc.tile_pool(name="econst", bufs=1) as ecpool,
                tc.tile_pool(name="eidx", bufs=2) as xpool,
                tc.tile_pool(name="egath", bufs=2) as gpool,
                tc.tile_pool(name="ework", bufs=4) as epool,
                tc.tile_pool(name="esel", bufs=4) as spool,
                tc.tile_pool(name="eout", bufs=2) as opool,
                tc.tile_pool(name="epsum", bufs=2, space="PSUM") as ep,
            ):
                iota_r = ecpool.tile([128, T, CN], bf16)
                nc.sync.dma_start(out=iota_r[:], in_=iotar_h.ap())
                biasb = ecpool.tile([128, HC], f32)
                nc.sync.dma_start(out=biasb[:], in_=biasb_h.ap())
                nC2e = ecpool.tile([128, 1], f32)
                nc.vector.memset(nC2e[:], -C2)

                for c in range(CHUNKS):
                    gidx_t = []
                    for cl in range(4):
                        gt = xpool.tile([128, TB[cl] * 8], i16, tag=f"g{cl}")
                        nc.sync.dma_start(out=gt[:], in_=gidx_h[cl].ap()[c])
                        gidx_t.append(gt)
                    nix = xpool.tile([128, 8], i16, tag="nix")
                    nc.sync.dma_start(out=nix[:], in_=nidx_h.ap()[c])
                    dl = xpool.tile([128, T], bf16, tag="dl")
                    nc.sync.dma_start(out=dl[:], in_=dstl_h.ap()[c])
                    own = xpool.tile([CN, HC], bf16, tag="own")
                    nc.sync.dma_start(
                        out=own[:], in_=ownxp_d[:][c * CN : (c + 1) * CN, :]
                    )

                    G = gpool.tile([128, T, XW], bf16)
                    toff = 0
                    for cl in range(4):
                        lo, hi = (0, SPLIT) if cl % 2 == 0 else (SPLIT, NPAD)
                        nc.gpsimd.dma_gather(
                            out_ap=G[:, toff : toff + TB[cl], :],
                            in_ap=xc_d[:][lo:hi, :],
                            idxs_ap=gidx_t[cl][:],
                            num_idxs=TB[cl] * 128,
                            num_idxs_reg=TB[cl] * 128,
                            elem_size=XW,
                            single_packet=False,
                        )
                        toff += TB[cl]
                    ADc = epool.tile([128, 1, HC], bf16, tag="ADc")
                    nc.gpsimd.dma_gather(
                        out_ap=ADc[:], in_ap=sc3_d[:], idxs_ap=nix[:],
                        num_idxs=128, num_idxs_reg=128, elem_size=HC,
                    )
                    adf = epool.tile([128, 1], f32, tag="adf")
                    nc.vector.tensor_tensor(
                        out=adf[:], in0=ADc[:, 0, 0:1], in1=ADc[:, 0, 1:2],
                        op=Alu.add,
                    )
                    B1 = epool.tile([128, 1], f32, tag="B1")
                    nc.scalar.activation(out=B1[:], in_=adf[:], func=Act.Exp,
                                         bias=nC2e[:], scale=1.0)
                    B2 = epool.tile([128, 1], f32, tag="B2")
                    nc.scalar.activation(out=B2[:], in_=adf[:], func=Act.Exp,
                                         bias=nC2e[:], scale=NEG)

                    S0a = spool.tile([128, T, CN], bf16, tag="S0a")
                    dl3 = dl[:].rearrange("p (t one) -> p t one", one=1)
                    nc.vector.tensor_tensor(
                        out=S0a[:], in0=dl3.to_broadcast([128, T, CN]),
                        in1=iota_r[:], op=Alu.is_equal,
                    )
                    p1 = ep.tile([CN, HC + 1], f32, tag="p1")
                    p2 = ep.tile([CN, HC + 1], f32, tag="p2")
                    for t in range(T):
                        pos = t < TPOS
                        ps = p1 if pos else p2
                        roff = 0 if pos else HC + 1
                        nc.tensor.matmul(
                            ps[:],
                            lhsT=S0a[:, t, :],
                            rhs=G[:, t, roff : roff + HC + 1],
                            start=(t == 0 or t == TPOS),
                            stop=(t == TPOS - 1 or t == T - 1),
                        )

                    n1 = opool.tile([CN, HC], f32, tag="n1")
                    nc.vector.tensor_scalar(
                        out=n1[:], in0=p1[:, 0:HC], scalar1=B1[0:CN, :],
                        scalar2=None, op0=Alu.mult,
                    )
                    n2 = opool.tile([CN, HC], f32, tag="n2")
                    nc.vector.tensor_scalar(
                        out=n2[:], in0=p2[:, 0:HC], scalar1=B2[0:CN, :],
                        scalar2=None, op0=Alu.mult,
                    )
                    agg = opool.tile([CN, HC], f32, tag="agg")
                    nc.vector.tensor_tensor(
                        out=agg[:], in0=n1[:], in1=n2[:], op=Alu.add
                    )
                    d1 = opool.tile([CN, 1], f32, tag="d1")
                    nc.vector.tensor_scalar(
                        out=d1[:], in0=p1[:, HC : HC + 1], scalar1=B1[0:CN, :],
                        scalar2=None, op0=Alu.mult,
                    )
                    d2 = opool.tile([CN, 1], f32, tag="d2")
                    nc.vector.tensor_scalar(
                        out=d2[:], in0=p2[:, HC : HC + 1], scalar1=B2[0:CN, :],
                        scalar2=EPS, op0=Alu.mult, op1=Alu.add,
                    )
                    den = opool.tile([CN, 1], f32, tag="den")
                    nc.vector.tensor_tensor(
                        out=den[:], in0=d1[:], in1=d2[:], op=Alu.add
                    )
                    rec = opool.tile([CN, 1], f32, tag="rec")
                    nc.vector.reciprocal(out=rec[:], in_=den[:])
                    o1 = opool.tile([CN, HC], f32, tag="o1")
                    nc.vector.tensor_scalar(
                        out=o1[:], in0=agg[:], scalar1=rec[:], scalar2=None,
                        op0=Alu.mult,
                    )
                    o2 = opool.tile([CN, HC], f32, tag="o2")
                    nc.vector.tensor_tensor(
                        out=o2[:], in0=o1[:], in1=own[:, 0:HC], op=Alu.add
                    )
                    o3 = opool.tile([CN, HC], f32, tag="o3")
                    nc.vector.tensor_tensor(
                        out=o3[:], in0=o2[:], in1=biasb[0:CN, :], op=Alu.add
                    )
                    nc.sync.dma_start(
                        out=out_h.ap()[c * CN : (c + 1) * CN, :], in_=o3[:]
                    )

    nc.compile()
    return nc


def _get_program(cfg, TB):
    key = (tuple(sorted(cfg.items())), TB)
    if key not in _PROG_CACHE:
        _PROG_CACHE[key] = build_program(cfg, TB)
    return _PROG_CACHE[key]


# ---------------------------------------------------------------- entry
def run(cfg, inputs, _profile=None):
    _ensure_ntff_hook()
    from concourse.bass_utils import run_bass_kernel_spmd

    in_maps, TB = host_prep(cfg, **inputs)
    nc = _get_program(cfg, TB)
    kwargs = {}
    if _profile is not None:
        kwargs = dict(trace=True, tmpdir=_profile)
    res = run_bass_kernel_spmd(
        nc, in_maps, core_ids=list(range(cfg["NCORES"])), **kwargs
    )
    out = np.concatenate(
        [res.results[k]["out"] for k in range(cfg["NCORES"])], axis=0
    )[: cfg["N"]]
    run.last_exec_time_ns = res.exec_time_ns
    return np.ascontiguousarray(out, dtype=np.float32)


def kernel(x, edge_index, v_mapping, W_src, att_src, att_dst, bias, _profile=None):
    inputs = dict(
        x=np.asarray(x, np.float32),
        edge_index=np.asarray(edge_index),
        v_mapping=np.asarray(v_mapping, np.float32),
        W_src=np.asarray(W_src, np.float32),
        att_src=np.asarray(att_src, np.float32),
        att_dst=np.asarray(att_dst, np.float32),
        bias=np.asarray(bias, np.float32),
    )
    out = run(FULL_CFG, inputs, _profile=_profile)
    kernel.last_exec_time_ns = run.last_exec_time_ns
    return out


# revision 19
# speedup vs baseline: 1.0917x; 1.0917x over previous
"""GAT attention-locator message passing on 8 Trainium2 NeuronCores.

Strategy: shard by destination node (N/8 dst nodes per core) so the
segment softmax is fully core-local.

Key factorization: with the global softmax shift C (valid per segment by
shift invariance) and the continuity of leaky_relu at 0, each edge weight
is  w = exp(as+ad-C)            if as+ad > 0
     w = exp(0.2(as+ad)-C)      otherwise
Both branches factor into (src term)x(dst term):
     w = A1[src]*B1[dst]  or  A2[src]*B2[dst]
  A1 = exp(as-C/2), B1 = exp(ad-C/2), A2 = exp(0.2as-C/2), B2 = exp(0.2ad-C/2).
The host classifies edges by sign(as+ad) (computed on CPU for LAYOUT only;
misclassification at the boundary is harmless since both branches agree
at 0).  The device then needs NO per-edge scalars at all:
 - A-terms are folded into pre-scaled gather tables (built on device):
     xc row m (bf16, 384 slots): [xp*A1 (128) | A1 | xp*A2 (128) | A2 | pad]
 - B-terms are applied per NODE after PSUM accumulation.
Per edge: one 768B dma_gather row + a shared 0/1 mask + one bf16 matmul
accumulating messages and the denominator (A column) into PSUM.

Per chunk (125 dst nodes): 4 gathers (sign x int16-index-range split),
one tiny per-node gather for ad (B-terms), T matmuls, per-node combine:
  out = (B1*psum1 + B2*psum2) / (B1*den1 + B2*den2 + eps) + x_proj + bias
"""

import math
import sys
import types

import numpy as np

# ------------------------------------------------------------ configuration
def make_cfg(N, E, F, HC, NCORES, CN, SPLIT, CSHIFT=12.0):
    NPC = N // NCORES
    assert NPC % CN == 0
    NPAD = ((N + 1023) // 1024) * 1024  # divisible by 8*128
    OWNK = (NPC + 127) // 128  # rows per partition in core-local tables
    cfg = dict(
        N=N, E=E, F=F, HC=HC, NCORES=NCORES, NPC=NPC, CN=CN,
        CHUNKS=NPC // CN, GCHUNKS=NCORES * (NPC // CN), SPLIT=SPLIT,
        NPAD=NPAD, NTILES=NPAD // 128, NGRP=NPAD // 1024,
        OWNK=OWNK, NOWN=OWNK * 128,
        CSHIFT=CSHIFT, EPS=1e-16, NEG=0.2,
    )
    assert cfg["NOWN"] <= NPAD
    return cfg


FULL_CFG = make_cfg(N=50000, E=1600000, F=256, HC=128, NCORES=8, CN=125,
                    SPLIT=32768)

_PROG_CACHE: dict = {}


def _ensure_ntff_hook():
    try:
        import antenv.axon_hooks  # noqa: F401

        return
    except ImportError:
        pass
    try:
        from trn_agent_boot.trn_boot import _ntff_profile_via_ctypes

        hook = _ntff_profile_via_ctypes("/opt/axon/libaxon_pjrt.so")
    except Exception:
        hook = None
    mod = types.ModuleType("antenv.axon_hooks")
    mod.get_axon_ntff_profile_hook = lambda: hook
    mod.set_axon_ntff_profile_hook = lambda h: None
    sys.modules["antenv.axon_hooks"] = mod


# ---------------------------------------------------------------- host prep
def _prep_edges(cfg, src, dst, possign):
    """4-way (sign, src-range) classed edge layout per dst chunk."""
    G, CN, SPLIT = cfg["GCHUNKS"], cfg["CN"], cfg["SPLIT"]
    src = src.astype(np.int64)
    dst = dst.astype(np.int64)
    g = dst // CN
    nl = dst % CN
    lowm = src < SPLIT
    # class: 0=pos-lo 1=pos-hi 2=neg-lo 3=neg-hi
    cls = np.where(possign, 0, 2) + np.where(lowm, 0, 1)

    key = g * 4 + cls
    order = np.argsort(key, kind="stable")
    g_s, nl_s, src_s, cls_s, key_s = (
        g[order], nl[order], src[order], cls[order], key[order]
    )
    starts = np.searchsorted(key_s, np.arange(4 * G))
    counts = np.bincount(key_s, minlength=4 * G)
    TB = [max(1, math.ceil(counts[c::4].max() / 128)) for c in range(4)]
    T = sum(TB)
    off = np.cumsum([0] + TB[:-1]) * 128  # class position offsets

    rank = np.arange(len(order)) - starts[key_s]
    pos = off[cls_s] + rank
    p = (pos % 128).astype(np.int64)
    t = (pos // 128).astype(np.int64)

    dstl = np.full((G, 128, T), -1.0, np.float32)
    dstl[g_s, p, t] = nl_s.astype(np.float32)

    idx = []
    for c in range(4):
        m = cls_s == c
        a = np.zeros((G, TB[c] * 128), np.int16)
        v = src_s[m] - (SPLIT if c % 2 else 0)
        a[g_s[m], rank[m]] = v.astype(np.int16)
        idx.append(a)

    def wrap(a):
        tb8 = a.shape[1] // 16
        w = a.reshape(G, tb8, 16).transpose(0, 2, 1)
        return np.ascontiguousarray(np.tile(w, (1, 8, 1)))

    return dict(TB=tuple(TB), T=T, dstl=dstl,
                idx=[wrap(a) for a in idx])


def host_prep(cfg, x, edge_index, v_mapping, W_src, att_src, att_dst, bias):
    N, F = cfg["N"], cfg["F"]
    NCORES, NPC, CN, CHUNKS = cfg["NCORES"], cfg["NPC"], cfg["CN"], cfg["CHUNKS"]
    NPAD, OWNK = cfg["NPAD"], cfg["OWNK"]

    def _fold(att):
        a = att[0]
        a = a / max(np.linalg.norm(a), 1e-12)
        return v_mapping.T @ a

    u_src = _fold(att_src)
    u_dst = _fold(att_dst)
    w_ext = np.concatenate(
        [W_src, u_src[:, None], u_dst[:, None]], axis=1
    ).astype(np.float32)

    x_pad = np.zeros((NPAD, F), np.float32)
    x_pad[:N] = x

    # sign classification (layout only; boundary-safe because the two
    # leaky-relu branches agree at 0)
    a_src_h = x @ u_src.astype(np.float32)
    a_dst_h = x @ u_dst.astype(np.float32)
    src, dst = edge_index[0].astype(np.int64), edge_index[1].astype(np.int64)
    possign = (a_src_h[src] + a_dst_h[dst]) > 0

    tabs = _prep_edges(cfg, src, dst, possign)

    import ml_dtypes
    iota_n = np.tile(np.arange(CN, dtype=np.float32), (128, 1))
    bias_b = np.tile(bias[None, :], (128, 1)).astype(np.float32)
    ident = np.eye(128, dtype=np.float32)

    # per-chunk per-node small-gather indices (core-independent)
    nidx = np.zeros((CHUNKS, 128), np.int16)
    for c in range(CHUNKS):
        nidx[c] = c * CN + np.minimum(np.arange(128), CN - 1)
    flat = nidx.reshape(-1)  # [CHUNKS*128]
    nidx_w = np.ascontiguousarray(
        np.tile(flat.reshape(CHUNKS * 8, 16).T, (8, 1))
    )  # [128, CHUNKS*8]

    T = tabs["T"]
    iota_rep = np.ascontiguousarray(
        np.broadcast_to(np.arange(CN, dtype=np.float32), (128, T, CN))
    ).astype(ml_dtypes.bfloat16)
    p128 = np.arange(128, dtype=np.int32)
    in_maps = []
    for k in range(NCORES):
        sl = slice(k * CHUNKS, (k + 1) * CHUNKS)
        base = k * NPC
        in_maps.append(
            {
                "x_pad": x_pad,
                "w_ext": w_ext,
                "iota_n": iota_n,
                "bias_b": bias_b,
                "ident": ident,
                "own_off": (base + p128 * OWNK).astype(np.int32)[:, None],
                "gidx0": tabs["idx"][0][sl],
                "gidx1": tabs["idx"][1][sl],
                "gidx2": tabs["idx"][2][sl],
                "gidx3": tabs["idx"][3][sl],
                "nidx": nidx_w,
                "iota_rep": iota_rep,
                "dstl": tabs["dstl"][sl].astype(ml_dtypes.bfloat16),
            }
        )
    return in_maps, tabs["TB"]


# ------------------------------------------------------------- bass program
def build_program(cfg, TB):
    import concourse.mybir as mybir
    import concourse.tile as tile
    from concourse import bacc
    from concourse.bass import IndirectOffsetOnAxis

    F, HC = cfg["F"], cfg["HC"]
    NCORES, CN, CHUNKS = cfg["NCORES"], cfg["CN"], cfg["CHUNKS"]
    NPC, NPAD, NTILES, NGRP = cfg["NPC"], cfg["NPAD"], cfg["NTILES"], cfg["NGRP"]
    SPLIT, OWNK, NOWN = cfg["SPLIT"], cfg["OWNK"], cfg["NOWN"]
    CSHIFT, EPS, NEG = cfg["CSHIFT"], cfg["EPS"], cfg["NEG"]
    KB = F // 128
    XW = 3 * HC  # combined row slots: [xpA1(128)|A1|xpA2(128)|A2|pad] in 384
    C2 = CSHIFT / 2.0

    T = sum(TB)
    TPOS = TB[0] + TB[1]
    f32 = mybir.dt.float32
    bf16 = mybir.dt.bfloat16
    i16 = mybir.dt.int16
    Alu = mybir.AluOpType
    Act = mybir.ActivationFunctionType

    nc = bacc.Bacc("TRN2", target_bir_lowering=False, debug=False, num_devices=NCORES)

    x_h = nc.dram_tensor("x_pad", [NPAD, F], f32, kind="ExternalInput")
    w_h = nc.dram_tensor("w_ext", [F, HC + 2], f32, kind="ExternalInput")
    iota_h = nc.dram_tensor("iota_n", [128, CN], f32, kind="ExternalInput")
    biasb_h = nc.dram_tensor("bias_b", [128, HC], f32, kind="ExternalInput")
    ident_h = nc.dram_tensor("ident", [128, 128], f32, kind="ExternalInput")
    ownoff_h = nc.dram_tensor("own_off", [128, 1], mybir.dt.int32, kind="ExternalInput")
    gidx_h = [
        nc.dram_tensor(f"gidx{c}", [CHUNKS, 128, TB[c] * 8], i16,
                       kind="ExternalInput")
        for c in range(4)
    ]
    nidx_h = nc.dram_tensor("nidx", [128, CHUNKS * 8], i16, kind="ExternalInput")
    iotar_h = nc.dram_tensor("iota_rep", [128, T, CN], bf16, kind="ExternalInput")
    dstl_h = nc.dram_tensor("dstl", [CHUNKS, 128, T], bf16, kind="ExternalInput")
    out_h = nc.dram_tensor("out", [NPC, HC], f32, kind="ExternalOutput")

    with tile.TileContext(nc) as tc:
        with tc.tile_pool(name="dram", bufs=1, space="DRAM") as dpool:
            xc_d = dpool.tile([NPAD, XW], bf16)     # combined scaled rows
            xr_d = dpool.tile([NPAD, HC], bf16)     # raw x_proj rows
            scT_d = dpool.tile([NPAD, 1], f32)      # a_dst node-contiguous
            sc3_d = dpool.tile([NOWN, HC], bf16)    # [ad_hi|ad_lo|junk] rows
            ownxp_d = dpool.tile([NOWN, HC], bf16)  # own raw x_proj rows

            # ---------------- Phase P: projection + tables ----------------
            with (
                tc.tile_pool(name="pconst", bufs=1) as cpool,
                tc.tile_pool(name="pio", bufs=3) as iopool,
                tc.tile_pool(name="pwork", bufs=4) as wpool,
                tc.tile_pool(name="ppsum", bufs=2, space="PSUM") as pp,
            ):
                wk = []
                for kb in range(KB):
                    wt = cpool.tile([128, HC + 2], f32, tag=f"w{kb}")
                    nc.sync.dma_start(
                        out=wt[:], in_=w_h.ap()[kb * 128 : (kb + 1) * 128, :]
                    )
                    wk.append(wt)
                ident = cpool.tile([128, 128], f32)
                nc.sync.dma_start(out=ident[:], in_=ident_h.ap())
                nC2p = cpool.tile([128, 1], f32)
                nc.vector.memset(nC2p[:], -C2)
                adst_sbuf = cpool.tile([128, NTILES], f32)

                x_r = x_h.ap().rearrange("(g a p) f -> g p a f", a=8, p=128)
                xc_r = xc_d[:].rearrange("(g a p) c -> g p a c", a=8, p=128)
                xr_r = xr_d[:].rearrange("(g a p) c -> g p a c", a=8, p=128)
                for gi in range(NGRP):
                    xt = iopool.tile([128, 8, F], f32)
                    nc.sync.dma_start(out=xt[:], in_=x_r[gi])
                    xec = iopool.tile([128, 8, XW], bf16, tag="xec")
                    nc.gpsimd.memset(xec[:, :, 2 * HC + 2 : XW], 0.0)
                    xer = iopool.tile([128, 8, HC], bf16, tag="xer")
                    for a in range(8):
                        nt = gi * 8 + a
                        px = pp.tile([128, HC + 2], f32, tag="px")
                        for kb in range(KB):
                            pt = pp.tile([128, 128], f32, tag=f"pt{kb}")
                            nc.tensor.transpose(
                                out=pt[:],
                                in_=xt[:, a, kb * 128 : (kb + 1) * 128],
                                identity=ident[:],
                            )
                            xT = wpool.tile([128, 128], f32, tag=f"xT{kb}")
                            nc.vector.tensor_copy(out=xT[:], in_=pt[:])
                            nc.tensor.matmul(
                                px[:],
                                lhsT=xT[:],
                                rhs=wk[kb][:],
                                start=(kb == 0),
                                stop=(kb == KB - 1),
                            )
                        A1f = wpool.tile([128, 1], f32, tag="A1f")
                        nc.scalar.activation(
                            out=A1f[:], in_=px[:, HC : HC + 1], func=Act.Exp,
                            bias=nC2p[:], scale=1.0,
                        )
                        A2f = wpool.tile([128, 1], f32, tag="A2f")
                        nc.scalar.activation(
                            out=A2f[:], in_=px[:, HC : HC + 1], func=Act.Exp,
                            bias=nC2p[:], scale=NEG,
                        )
                        nc.vector.tensor_scalar(
                            out=xec[:, a, 0:HC], in0=px[:, 0:HC], scalar1=A1f[:],
                            scalar2=None, op0=Alu.mult,
                        )
                        nc.gpsimd.tensor_copy(
                            out=xec[:, a, HC : HC + 1], in_=A1f[:]
                        )
                        nc.vector.tensor_scalar(
                            out=xec[:, a, HC + 1 : 2 * HC + 1], in0=px[:, 0:HC],
                            scalar1=A2f[:], scalar2=None, op0=Alu.mult,
                        )
                        nc.gpsimd.tensor_copy(
                            out=xec[:, a, 2 * HC + 1 : 2 * HC + 2], in_=A2f[:]
                        )
                        nc.vector.tensor_copy(out=xer[:, a, :], in_=px[:, 0:HC])
                        nc.vector.tensor_copy(
                            out=adst_sbuf[:, nt : nt + 1],
                            in_=px[:, HC + 1 : HC + 2],
                        )
                    nc.sync.dma_start(out=xc_r[gi], in_=xec[:])
                    nc.sync.dma_start(out=xr_r[gi], in_=xer[:])
                scT_r = scT_d[:].rearrange("(nt p) one -> p (nt one)", p=128)
                nc.sync.dma_start(out=scT_r, in_=adst_sbuf[:])

                ownoff = cpool.tile([128, 1], mybir.dt.int32)
                nc.sync.dma_start(out=ownoff[:], in_=ownoff_h.ap())
                own_ad = cpool.tile([128, OWNK], f32)
                nc.gpsimd.indirect_dma_start(
                    out=own_ad[:], out_offset=None, in_=scT_d[:],
                    in_offset=IndirectOffsetOnAxis(ap=ownoff[:], axis=0),
                )
                sc3s = cpool.tile([128, OWNK, HC], bf16)
                nc.gpsimd.memset(sc3s[:], 0.0)
                nc.vector.tensor_copy(out=sc3s[:, :, 0], in_=own_ad[:])
                nc.vector.tensor_tensor(
                    out=sc3s[:, :, 1], in0=own_ad[:], in1=sc3s[:, :, 0],
                    op=Alu.subtract,
                )
                sc3_r = sc3_d[:].rearrange("(p i) c -> p i c", p=128)
                nc.sync.dma_start(out=sc3_r, in_=sc3s[:])
                own_xe = cpool.tile([128, OWNK * HC], bf16)
                nc.gpsimd.indirect_dma_start(
                    out=own_xe[:], out_offset=None, in_=xr_d[:],
                    in_offset=IndirectOffsetOnAxis(ap=ownoff[:], axis=0),
                )
                oxp_r = ownxp_d[:].rearrange("(p i) c -> p (i c)", p=128)
                nc.sync.dma_start(out=oxp_r, in_=own_xe[:])

            # ---------------- Phase E: edges ------------------------------
            with (
                tc.tile_pool(name="econst", bufs=1) as ecpool,
                tc.tile_pool(name="eidx", bufs=2) as xpool,
                tc.tile_pool(name="egath", bufs=3) as gpool,
                tc.tile_pool(name="ework", bufs=4) as epool,
                tc.tile_pool(name="esel", bufs=4) as spool,
                tc.tile_pool(name="eout", bufs=2) as opool,
                tc.tile_pool(name="epsum", bufs=3, space="PSUM") as ep,
            ):
                iota_r = ecpool.tile([128, T, CN], bf16)
                nc.sync.dma_start(out=iota_r[:], in_=iotar_h.ap())
                nixall = ecpool.tile([128, CHUNKS * 8], i16)
                nc.sync.dma_start(out=nixall[:], in_=nidx_h.ap())
                ADall = ecpool.tile([128, CHUNKS, HC], bf16)
                nc.gpsimd.dma_gather(
                    out_ap=ADall[:], in_ap=sc3_d[:], idxs_ap=nixall[:],
                    num_idxs=CHUNKS * 128, num_idxs_reg=CHUNKS * 128,
                    elem_size=HC, single_packet=False,
                )
                biasb = ecpool.tile([128, HC], f32)
                nc.sync.dma_start(out=biasb[:], in_=biasb_h.ap())
                nC2e = ecpool.tile([128, 1], f32)
                nc.vector.memset(nC2e[:], -C2)

                for c in range(CHUNKS):
                    gidx_t = []
                    for cl in range(4):
                        gt = xpool.tile([128, TB[cl] * 8], i16, tag=f"g{cl}")
                        nc.sync.dma_start(out=gt[:], in_=gidx_h[cl].ap()[c])
                        gidx_t.append(gt)
                    dl = xpool.tile([128, T], bf16, tag="dl")
                    nc.sync.dma_start(out=dl[:], in_=dstl_h.ap()[c])
                    own = xpool.tile([CN, HC], bf16, tag="own")
                    nc.sync.dma_start(
                        out=own[:], in_=ownxp_d[:][c * CN : (c + 1) * CN, :]
                    )

                    G = gpool.tile([128, T, XW], bf16)
                    toff = 0
                    for cl in range(4):
                        lo, hi = (0, SPLIT) if cl % 2 == 0 else (SPLIT, NPAD)
                        nc.gpsimd.dma_gather(
                            out_ap=G[:, toff : toff + TB[cl], :],
                            in_ap=xc_d[:][lo:hi, :],
                            idxs_ap=gidx_t[cl][:],
                            num_idxs=TB[cl] * 128,
                            num_idxs_reg=TB[cl] * 128,
                            elem_size=XW,
                            single_packet=False,
                        )
                        toff += TB[cl]
                    adf = epool.tile([128, 1], f32, tag="adf")
                    nc.vector.tensor_tensor(
                        out=adf[:], in0=ADall[:, c, 0:1], in1=ADall[:, c, 1:2],
                        op=Alu.add,
                    )
                    B1 = epool.tile([128, 1], f32, tag="B1")
                    nc.scalar.activation(out=B1[:], in_=adf[:], func=Act.Exp,
                                         bias=nC2e[:], scale=1.0)
                    B2 = epool.tile([128, 1], f32, tag="B2")
                    nc.scalar.activation(out=B2[:], in_=adf[:], func=Act.Exp,
                                         bias=nC2e[:], scale=NEG)

                    S0a = spool.tile([128, T, CN], bf16, tag="S0a")
                    dl3 = dl[:].rearrange("p (t one) -> p t one", one=1)
                    nc.vector.tensor_tensor(
                        out=S0a[:], in0=dl3.to_broadcast([128, T, CN]),
                        in1=iota_r[:], op=Alu.is_equal,
                    )
                    p1 = ep.tile([CN, HC + 1], f32, tag="p1")
                    p2 = ep.tile([CN, HC + 1], f32, tag="p2")
                    for t in range(T):
                        pos = t < TPOS
                        ps = p1 if pos else p2
                        roff = 0 if pos else HC + 1
                        nc.tensor.matmul(
                            ps[:],
                            lhsT=S0a[:, t, :],
                            rhs=G[:, t, roff : roff + HC + 1],
                            start=(t == 0 or t == TPOS),
                            stop=(t == TPOS - 1 or t == T - 1),
                        )

                    n1 = opool.tile([CN, HC], f32, tag="n1")
                    nc.vector.tensor_scalar(
                        out=n1[:], in0=p1[:, 0:HC], scalar1=B1[0:CN, :],
                        scalar2=None, op0=Alu.mult,
                    )
                    n2 = opool.tile([CN, HC], f32, tag="n2")
                    nc.vector.tensor_scalar(
                        out=n2[:], in0=p2[:, 0:HC], scalar1=B2[0:CN, :],
                        scalar2=None, op0=Alu.mult,
                    )
                    agg = opool.tile([CN, HC], f32, tag="agg")
                    nc.vector.tensor_tensor(
                        out=agg[:], in0=n1[:], in1=n2[:], op=Alu.add
                    )
                    d1 = opool.tile([CN, 1], f32, tag="d1")
                    nc.vector.tensor_scalar(
                        out=d1[:], in0=p1[:, HC : HC + 1], scalar1=B1[0:CN, :],
                        scalar2=None, op0=Alu.mult,
                    )
                    d2 = opool.tile([CN, 1], f32, tag="d2")
                    nc.vector.tensor_scalar(
                        out=d2[:], in0=p2[:, HC : HC + 1], scalar1=B2[0:CN, :],
                        scalar2=EPS, op0=Alu.mult, op1=Alu.add,
                    )
                    den = opool.tile([CN, 1], f32, tag="den")
                    nc.vector.tensor_tensor(
                        out=den[:], in0=d1[:], in1=d2[:], op=Alu.add
                    )
                    rec = opool.tile([CN, 1], f32, tag="rec")
                    nc.vector.reciprocal(out=rec[:], in_=den[:])
                    o1 = opool.tile([CN, HC], f32, tag="o1")
                    nc.vector.tensor_scalar(
                        out=o1[:], in0=agg[:], scalar1=rec[:], scalar2=None,
                        op0=Alu.mult,
                    )
                    o2 = opool.tile([CN, HC], f32, tag="o2")
                    nc.vector.tensor_tensor(
                        out=o2[:], in0=o1[:], in1=own[:, 0:HC], op=Alu.add
                    )
                    o3 = opool.tile([CN, HC], f32, tag="o3")
                    nc.vector.tensor_tensor(
                        out=o3[:], in0=o2[:], in1=biasb[0:CN, :], op=Alu.add
                    )
                    nc.sync.dma_start(
                        out=out_h.ap()[c * CN : (c + 1) * CN, :], in_=o3[:]
                    )

    nc.compile()
    return nc


def _get_program(cfg, TB):
    key = (tuple(sorted(cfg.items())), TB)
    if key not in _PROG_CACHE:
        _PROG_CACHE[key] = build_program(cfg, TB)
    return _PROG_CACHE[key]


# ---------------------------------------------------------------- entry
def run(cfg, inputs, _profile=None):
    _ensure_ntff_hook()
    from concourse.bass_utils import run_bass_kernel_spmd

    in_maps, TB = host_prep(cfg, **inputs)
    nc = _get_program(cfg, TB)
    kwargs = {}
    if _profile is not None:
        kwargs = dict(trace=True, tmpdir=_profile)
    res = run_bass_kernel_spmd(
        nc, in_maps, core_ids=list(range(cfg["NCORES"])), **kwargs
    )
    out = np.concatenate(
        [res.results[k]["out"] for k in range(cfg["NCORES"])], axis=0
    )[: cfg["N"]]
    run.last_exec_time_ns = res.exec_time_ns
    return np.ascontiguousarray(out, dtype=np.float32)


def kernel(x, edge_index, v_mapping, W_src, att_src, att_dst, bias, _profile=None):
    inputs = dict(
        x=np.asarray(x, np.float32),
        edge_index=np.asarray(edge_index),
        v_mapping=np.asarray(v_mapping, np.float32),
        W_src=np.asarray(W_src, np.float32),
        att_src=np.asarray(att_src, np.float32),
        att_dst=np.asarray(att_dst, np.float32),
        bias=np.asarray(bias, np.float32),
    )
    out = run(FULL_CFG, inputs, _profile=_profile)
    kernel.last_exec_time_ns = run.last_exec_time_ns
    return out


# revision 21
# speedup vs baseline: 1.1396x; 1.0439x over previous
"""GAT attention-locator message passing on 8 Trainium2 NeuronCores.

Strategy: shard by destination node (N/8 dst nodes per core) so the
segment softmax is fully core-local.

Key factorization: with the global softmax shift C (valid per segment by
shift invariance) and the continuity of leaky_relu at 0, each edge weight
is  w = exp(as+ad-C)            if as+ad > 0
     w = exp(0.2(as+ad)-C)      otherwise
Both branches factor into (src term)x(dst term):
     w = A1[src]*B1[dst]  or  A2[src]*B2[dst]
  A1 = exp(as-C/2), B1 = exp(ad-C/2), A2 = exp(0.2as-C/2), B2 = exp(0.2ad-C/2).
The host classifies edges by sign(as+ad) (computed on CPU for LAYOUT only;
misclassification at the boundary is harmless since both branches agree
at 0).  The device then needs NO per-edge scalars at all:
 - A-terms are folded into pre-scaled gather tables (built on device):
     xc row m (bf16, 384 slots): [xp*A1 (128) | A1 | xp*A2 (128) | A2 | pad]
 - B-terms are applied per NODE after PSUM accumulation.
Per edge: one 768B dma_gather row + a shared 0/1 mask + one bf16 matmul
accumulating messages and the denominator (A column) into PSUM.

Per chunk (125 dst nodes): 4 gathers (sign x int16-index-range split),
one tiny per-node gather for ad (B-terms), T matmuls, per-node combine:
  out = (B1*psum1 + B2*psum2) / (B1*den1 + B2*den2 + eps) + x_proj + bias
"""

import math
import sys
import types

import numpy as np

# ------------------------------------------------------------ configuration
def make_cfg(N, E, F, HC, NCORES, CN, SPLIT, CSHIFT=12.0):
    NPC = N // NCORES
    assert NPC % CN == 0
    NPAD = ((N + 1023) // 1024) * 1024  # divisible by 8*128
    OWNK = (NPC + 127) // 128  # rows per partition in core-local tables
    cfg = dict(
        N=N, E=E, F=F, HC=HC, NCORES=NCORES, NPC=NPC, CN=CN,
        CHUNKS=NPC // CN, GCHUNKS=NCORES * (NPC // CN), SPLIT=SPLIT,
        NPAD=NPAD, NTILES=NPAD // 128, NGRP=NPAD // 1024,
        OWNK=OWNK, NOWN=OWNK * 128,
        CSHIFT=CSHIFT, EPS=1e-16, NEG=0.2,
    )
    assert cfg["NOWN"] <= NPAD
    return cfg


FULL_CFG = make_cfg(N=50000, E=1600000, F=256, HC=128, NCORES=8, CN=125,
                    SPLIT=32768)

_PROG_CACHE: dict = {}


def _patch_ldw_opt():
    """Enable walrus LDWEIGHTS pipelining (merges weight loads into the
    matmul stream); the default pipeline disables it."""
    import concourse.bass_utils as bu

    if getattr(bu, "_ldw_patched", False):
        return
    orig = bu.run_command

    def patched(argv, **kw):
        argv = [a.replace("--enable-ldw-opt=false", "--enable-ldw-opt=true")
                if isinstance(a, str) else a for a in argv]
        return orig(argv, **kw)

    bu.run_command = patched
    bu._ldw_patched = True


def _ensure_ntff_hook():
    try:
        import antenv.axon_hooks  # noqa: F401

        return
    except ImportError:
        pass
    try:
        from trn_agent_boot.trn_boot import _ntff_profile_via_ctypes

        hook = _ntff_profile_via_ctypes("/opt/axon/libaxon_pjrt.so")
    except Exception:
        hook = None
    mod = types.ModuleType("antenv.axon_hooks")
    mod.get_axon_ntff_profile_hook = lambda: hook
    mod.set_axon_ntff_profile_hook = lambda h: None
    sys.modules["antenv.axon_hooks"] = mod


# ---------------------------------------------------------------- host prep
def _prep_edges(cfg, src, dst, possign):
    """4-way (sign, src-range) classed edge layout per dst chunk."""
    G, CN, SPLIT = cfg["GCHUNKS"], cfg["CN"], cfg["SPLIT"]
    src = src.astype(np.int64)
    dst = dst.astype(np.int64)
    g = dst // CN
    nl = dst % CN
    lowm = src < SPLIT
    # class: 0=pos-lo 1=pos-hi 2=neg-lo 3=neg-hi
    cls = np.where(possign, 0, 2) + np.where(lowm, 0, 1)

    key = (g * 4 + cls) * 65536 + src  # src-sorted within class: HBM locality
    order = np.argsort(key, kind="stable")
    key = key // 65536
    g_s, nl_s, src_s, cls_s = g[order], nl[order], src[order], cls[order]
    key_s = key[order]
    starts = np.searchsorted(key_s, np.arange(4 * G))
    counts = np.bincount(key_s, minlength=4 * G)
    TB = [max(1, math.ceil(counts[c::4].max() / 128)) for c in range(4)]
    T = sum(TB)
    off = np.cumsum([0] + TB[:-1]) * 128  # class position offsets

    rank = np.arange(len(order)) - starts[key_s]
    pos = off[cls_s] + rank
    p = (pos % 128).astype(np.int64)
    t = (pos // 128).astype(np.int64)

    dstl = np.full((G, 128, T), -1.0, np.float32)
    dstl[g_s, p, t] = nl_s.astype(np.float32)

    idx = []
    for c in range(4):
        m = cls_s == c
        a = np.zeros((G, TB[c] * 128), np.int16)
        v = src_s[m] - (SPLIT if c % 2 else 0)
        a[g_s[m], rank[m]] = v.astype(np.int16)
        idx.append(a)

    def wrap(a):
        tb8 = a.shape[1] // 16
        w = a.reshape(G, tb8, 16).transpose(0, 2, 1)
        return np.ascontiguousarray(np.tile(w, (1, 8, 1)))

    return dict(TB=tuple(TB), T=T, dstl=dstl,
                idx=[wrap(a) for a in idx])


def host_prep(cfg, x, edge_index, v_mapping, W_src, att_src, att_dst, bias):
    N, F = cfg["N"], cfg["F"]
    NCORES, NPC, CN, CHUNKS = cfg["NCORES"], cfg["NPC"], cfg["CN"], cfg["CHUNKS"]
    NPAD, OWNK = cfg["NPAD"], cfg["OWNK"]

    def _fold(att):
        a = att[0]
        a = a / max(np.linalg.norm(a), 1e-12)
        return v_mapping.T @ a

    u_src = _fold(att_src)
    u_dst = _fold(att_dst)
    w_ext = np.concatenate(
        [W_src, u_src[:, None], u_dst[:, None]], axis=1
    ).astype(np.float32)

    x_pad = np.zeros((NPAD, F), np.float32)
    x_pad[:N] = x

    # sign classification (layout only; boundary-safe because the two
    # leaky-relu branches agree at 0)
    a_src_h = x @ u_src.astype(np.float32)
    a_dst_h = x @ u_dst.astype(np.float32)
    src, dst = edge_index[0].astype(np.int64), edge_index[1].astype(np.int64)
    possign = (a_src_h[src] + a_dst_h[dst]) > 0

    tabs = _prep_edges(cfg, src, dst, possign)

    import ml_dtypes
    iota_n = np.tile(np.arange(CN, dtype=np.float32), (128, 1))
    bias_b = np.tile(bias[None, :], (128, 1)).astype(np.float32)
    ident = np.eye(128, dtype=np.float32)

    # per-chunk per-node small-gather indices (core-independent)
    nidx = np.zeros((CHUNKS, 128), np.int16)
    for c in range(CHUNKS):
        nidx[c] = c * CN + np.minimum(np.arange(128), CN - 1)
    flat = nidx.reshape(-1)  # [CHUNKS*128]
    nidx_w = np.ascontiguousarray(
        np.tile(flat.reshape(CHUNKS * 8, 16).T, (8, 1))
    )  # [128, CHUNKS*8]

    T = tabs["T"]
    iota_rep = np.ascontiguousarray(
        np.broadcast_to(np.arange(CN, dtype=np.float32), (128, T, CN))
    ).astype(ml_dtypes.bfloat16)
    p128 = np.arange(128, dtype=np.int32)
    in_maps = []
    for k in range(NCORES):
        sl = slice(k * CHUNKS, (k + 1) * CHUNKS)
        base = k * NPC
        in_maps.append(
            {
                "x_pad": x_pad,
                "w_ext": w_ext,
                "iota_n": iota_n,
                "bias_b": bias_b,
                "ident": ident,
                "own_off": (base + p128 * OWNK).astype(np.int32)[:, None],
                "gidx0": tabs["idx"][0][sl],
                "gidx1": tabs["idx"][1][sl],
                "gidx2": tabs["idx"][2][sl],
                "gidx3": tabs["idx"][3][sl],
                "nidx": nidx_w,
                "iota_rep": iota_rep,
                "dstl": tabs["dstl"][sl].astype(ml_dtypes.bfloat16),
            }
        )
    return in_maps, tabs["TB"]


# ------------------------------------------------------------- bass program
def build_program(cfg, TB):
    import concourse.mybir as mybir
    import concourse.tile as tile
    from concourse import bacc
    from concourse.bass import IndirectOffsetOnAxis

    F, HC = cfg["F"], cfg["HC"]
    NCORES, CN, CHUNKS = cfg["NCORES"], cfg["CN"], cfg["CHUNKS"]
    NPC, NPAD, NTILES, NGRP = cfg["NPC"], cfg["NPAD"], cfg["NTILES"], cfg["NGRP"]
    SPLIT, OWNK, NOWN = cfg["SPLIT"], cfg["OWNK"], cfg["NOWN"]
    CSHIFT, EPS, NEG = cfg["CSHIFT"], cfg["EPS"], cfg["NEG"]
    KB = F // 128
    XW = 2 * HC  # table row slots: [xp*A (128) | A | junk] in 256 (512B)
    C2 = CSHIFT / 2.0

    T = sum(TB)
    TPOS = TB[0] + TB[1]
    f32 = mybir.dt.float32
    bf16 = mybir.dt.bfloat16
    i16 = mybir.dt.int16
    Alu = mybir.AluOpType
    Act = mybir.ActivationFunctionType

    nc = bacc.Bacc("TRN2", target_bir_lowering=False, debug=False, num_devices=NCORES)

    x_h = nc.dram_tensor("x_pad", [NPAD, F], f32, kind="ExternalInput")
    w_h = nc.dram_tensor("w_ext", [F, HC + 2], f32, kind="ExternalInput")
    iota_h = nc.dram_tensor("iota_n", [128, CN], f32, kind="ExternalInput")
    biasb_h = nc.dram_tensor("bias_b", [128, HC], f32, kind="ExternalInput")
    ident_h = nc.dram_tensor("ident", [128, 128], f32, kind="ExternalInput")
    ownoff_h = nc.dram_tensor("own_off", [128, 1], mybir.dt.int32, kind="ExternalInput")
    gidx_h = [
        nc.dram_tensor(f"gidx{c}", [CHUNKS, 128, TB[c] * 8], i16,
                       kind="ExternalInput")
        for c in range(4)
    ]
    nidx_h = nc.dram_tensor("nidx", [128, CHUNKS * 8], i16, kind="ExternalInput")
    iotar_h = nc.dram_tensor("iota_rep", [128, T, CN], bf16, kind="ExternalInput")
    dstl_h = nc.dram_tensor("dstl", [CHUNKS, 128, T], bf16, kind="ExternalInput")
    out_h = nc.dram_tensor("out", [NPC, HC], f32, kind="ExternalOutput")

    with tile.TileContext(nc) as tc:
        with tc.tile_pool(name="dram", bufs=1, space="DRAM") as dpool:
            xp_t = dpool.tile([NPAD, XW], bf16)     # pos-scaled rows
            xn_t = dpool.tile([NPAD, XW], bf16)     # neg-scaled rows
            xr_d = dpool.tile([NPAD, HC], bf16)     # raw x_proj rows
            scT_d = dpool.tile([NPAD, 1], f32)      # a_dst node-contiguous
            sc3_d = dpool.tile([NOWN, HC], bf16)    # [ad_hi|ad_lo|junk] rows
            ownxp_d = dpool.tile([NOWN, HC], bf16)  # own raw x_proj rows

            # ---------------- Phase P: projection + tables ----------------
            with (
                tc.tile_pool(name="pconst", bufs=1) as cpool,
                tc.tile_pool(name="pio", bufs=3) as iopool,
                tc.tile_pool(name="pwork", bufs=4) as wpool,
                tc.tile_pool(name="ppsum", bufs=2, space="PSUM") as pp,
            ):
                wk = []
                for kb in range(KB):
                    wt = cpool.tile([128, HC + 2], f32, tag=f"w{kb}")
                    nc.sync.dma_start(
                        out=wt[:], in_=w_h.ap()[kb * 128 : (kb + 1) * 128, :]
                    )
                    wk.append(wt)
                ident = cpool.tile([128, 128], f32)
                nc.sync.dma_start(out=ident[:], in_=ident_h.ap())
                nC2p = cpool.tile([128, 1], f32)
                nc.vector.memset(nC2p[:], -C2)
                adst_sbuf = cpool.tile([128, NTILES], f32)

                x_r = x_h.ap().rearrange("(g a p) f -> g p a f", a=8, p=128)
                xp_r = xp_t[:].rearrange("(g a p) c -> g p a c", a=8, p=128)
                xn_r = xn_t[:].rearrange("(g a p) c -> g p a c", a=8, p=128)
                xr_r = xr_d[:].rearrange("(g a p) c -> g p a c", a=8, p=128)
                for gi in range(NGRP):
                    xt = iopool.tile([128, 8, F], f32)
                    nc.sync.dma_start(out=xt[:], in_=x_r[gi])
                    xep = iopool.tile([128, 8, XW], bf16, tag="xep")
                    nc.gpsimd.memset(xep[:, :, HC + 1 : XW], 0.0)
                    xen = iopool.tile([128, 8, XW], bf16, tag="xen")
                    nc.gpsimd.memset(xen[:, :, HC + 1 : XW], 0.0)
                    xer = iopool.tile([128, 8, HC], bf16, tag="xer")
                    for a in range(8):
                        nt = gi * 8 + a
                        px = pp.tile([128, HC + 2], f32, tag="px")
                        for kb in range(KB):
                            pt = pp.tile([128, 128], f32, tag=f"pt{kb}")
                            nc.tensor.transpose(
                                out=pt[:],
                                in_=xt[:, a, kb * 128 : (kb + 1) * 128],
                                identity=ident[:],
                            )
                            xT = wpool.tile([128, 128], f32, tag=f"xT{kb}")
                            nc.vector.tensor_copy(out=xT[:], in_=pt[:])
                            nc.tensor.matmul(
                                px[:],
                                lhsT=xT[:],
                                rhs=wk[kb][:],
                                start=(kb == 0),
                                stop=(kb == KB - 1),
                            )
                        A1f = wpool.tile([128, 1], f32, tag="A1f")
                        nc.scalar.activation(
                            out=A1f[:], in_=px[:, HC : HC + 1], func=Act.Exp,
                            bias=nC2p[:], scale=1.0,
                        )
                        A2f = wpool.tile([128, 1], f32, tag="A2f")
                        nc.scalar.activation(
                            out=A2f[:], in_=px[:, HC : HC + 1], func=Act.Exp,
                            bias=nC2p[:], scale=NEG,
                        )
                        nc.vector.tensor_scalar(
                            out=xep[:, a, 0:HC], in0=px[:, 0:HC], scalar1=A1f[:],
                            scalar2=None, op0=Alu.mult,
                        )
                        nc.gpsimd.tensor_copy(
                            out=xep[:, a, HC : HC + 1], in_=A1f[:]
                        )
                        nc.vector.tensor_scalar(
                            out=xen[:, a, 0:HC], in0=px[:, 0:HC],
                            scalar1=A2f[:], scalar2=None, op0=Alu.mult,
                        )
                        nc.gpsimd.tensor_copy(
                            out=xen[:, a, HC : HC + 1], in_=A2f[:]
                        )
                        nc.vector.tensor_copy(out=xer[:, a, :], in_=px[:, 0:HC])
                        nc.vector.tensor_copy(
                            out=adst_sbuf[:, nt : nt + 1],
                            in_=px[:, HC + 1 : HC + 2],
                        )
                    nc.sync.dma_start(out=xp_r[gi], in_=xep[:])
                    nc.sync.dma_start(out=xn_r[gi], in_=xen[:])
                    nc.sync.dma_start(out=xr_r[gi], in_=xer[:])
                scT_r = scT_d[:].rearrange("(nt p) one -> p (nt one)", p=128)
                nc.sync.dma_start(out=scT_r, in_=adst_sbuf[:])

                ownoff = cpool.tile([128, 1], mybir.dt.int32)
                nc.sync.dma_start(out=ownoff[:], in_=ownoff_h.ap())
                own_ad = cpool.tile([128, OWNK], f32)
                nc.gpsimd.indirect_dma_start(
                    out=own_ad[:], out_offset=None, in_=scT_d[:],
                    in_offset=IndirectOffsetOnAxis(ap=ownoff[:], axis=0),
                )
                sc3s = cpool.tile([128, OWNK, HC], bf16)
                nc.gpsimd.memset(sc3s[:], 0.0)
                nc.vector.tensor_copy(out=sc3s[:, :, 0], in_=own_ad[:])
                nc.vector.tensor_tensor(
                    out=sc3s[:, :, 1], in0=own_ad[:], in1=sc3s[:, :, 0],
                    op=Alu.subtract,
                )
                sc3_r = sc3_d[:].rearrange("(p i) c -> p i c", p=128)
                nc.sync.dma_start(out=sc3_r, in_=sc3s[:])
                own_xe = cpool.tile([128, OWNK * HC], bf16)
                nc.gpsimd.indirect_dma_start(
                    out=own_xe[:], out_offset=None, in_=xr_d[:],
                    in_offset=IndirectOffsetOnAxis(ap=ownoff[:], axis=0),
                )
                oxp_r = ownxp_d[:].rearrange("(p i) c -> p (i c)", p=128)
                nc.sync.dma_start(out=oxp_r, in_=own_xe[:])

            # ---------------- Phase E: edges ------------------------------
            with (
                tc.tile_pool(name="econst", bufs=1) as ecpool,
                tc.tile_pool(name="eidx", bufs=2) as xpool,
                tc.tile_pool(name="egath", bufs=3) as gpool,
                tc.tile_pool(name="ework", bufs=4) as epool,
                tc.tile_pool(name="esel", bufs=4) as spool,
                tc.tile_pool(name="eout", bufs=2) as opool,
                tc.tile_pool(name="epsum", bufs=3, space="PSUM") as ep,
            ):
                iota_r = ecpool.tile([128, T, CN], bf16)
                nc.sync.dma_start(out=iota_r[:], in_=iotar_h.ap())
                nixall = ecpool.tile([128, CHUNKS * 8], i16)
                nc.sync.dma_start(out=nixall[:], in_=nidx_h.ap())
                ADall = ecpool.tile([128, CHUNKS, HC], bf16)
                nc.gpsimd.dma_gather(
                    out_ap=ADall[:], in_ap=sc3_d[:], idxs_ap=nixall[:],
                    num_idxs=CHUNKS * 128, num_idxs_reg=CHUNKS * 128,
                    elem_size=HC, single_packet=False,
                )
                biasb = ecpool.tile([128, HC], f32)
                nc.sync.dma_start(out=biasb[:], in_=biasb_h.ap())
                nC2e = ecpool.tile([128, 1], f32)
                nc.vector.memset(nC2e[:], -C2)

                for c in range(CHUNKS):
                    gidx_t = []
                    for cl in range(4):
                        gt = xpool.tile([128, TB[cl] * 8], i16, tag=f"g{cl}")
                        nc.sync.dma_start(out=gt[:], in_=gidx_h[cl].ap()[c])
                        gidx_t.append(gt)
                    dl = xpool.tile([128, T], bf16, tag="dl")
                    nc.sync.dma_start(out=dl[:], in_=dstl_h.ap()[c])
                    own = xpool.tile([CN, HC], bf16, tag="own")
                    nc.sync.dma_start(
                        out=own[:], in_=ownxp_d[:][c * CN : (c + 1) * CN, :]
                    )

                    G = gpool.tile([128, T, XW], bf16)
                    toff = 0
                    for cl in range(4):
                        lo, hi = (0, SPLIT) if cl % 2 == 0 else (SPLIT, NPAD)
                        tab = xp_t if cl < 2 else xn_t
                        nc.gpsimd.dma_gather(
                            out_ap=G[:, toff : toff + TB[cl], :],
                            in_ap=tab[:][lo:hi, :],
                            idxs_ap=gidx_t[cl][:],
                            num_idxs=TB[cl] * 128,
                            num_idxs_reg=TB[cl] * 128,
                            elem_size=XW,
                            single_packet=False,
                        )
                        toff += TB[cl]
                    adf = epool.tile([128, 1], f32, tag="adf")
                    nc.vector.tensor_tensor(
                        out=adf[:], in0=ADall[:, c, 0:1], in1=ADall[:, c, 1:2],
                        op=Alu.add,
                    )
                    B1 = epool.tile([128, 1], f32, tag="B1")
                    nc.scalar.activation(out=B1[:], in_=adf[:], func=Act.Exp,
                                         bias=nC2e[:], scale=1.0)
                    B2 = epool.tile([128, 1], f32, tag="B2")
                    nc.scalar.activation(out=B2[:], in_=adf[:], func=Act.Exp,
                                         bias=nC2e[:], scale=NEG)

                    S0a = spool.tile([128, T, CN], bf16, tag="S0a")
                    dl3 = dl[:].rearrange("p (t one) -> p t one", one=1)
                    nc.vector.tensor_tensor(
                        out=S0a[:], in0=dl3.to_broadcast([128, T, CN]),
                        in1=iota_r[:], op=Alu.is_equal,
                    )
                    p1 = ep.tile([CN, HC + 1], f32, tag="p1")
                    p2 = ep.tile([CN, HC + 1], f32, tag="p2")
                    for t in range(T):
                        pos = t < TPOS
                        ps = p1 if pos else p2
                        nc.tensor.matmul(
                            ps[:],
                            lhsT=S0a[:, t, :],
                            rhs=G[:, t, 0 : HC + 1],
                            start=(t == 0 or t == TPOS),
                            stop=(t == TPOS - 1 or t == T - 1),
                        )

                    n1 = opool.tile([CN, HC], f32, tag="n1")
                    nc.vector.tensor_scalar(
                        out=n1[:], in0=p1[:, 0:HC], scalar1=B1[0:CN, :],
                        scalar2=None, op0=Alu.mult,
                    )
                    n2 = opool.tile([CN, HC], f32, tag="n2")
                    nc.vector.tensor_scalar(
                        out=n2[:], in0=p2[:, 0:HC], scalar1=B2[0:CN, :],
                        scalar2=None, op0=Alu.mult,
                    )
                    agg = opool.tile([CN, HC], f32, tag="agg")
                    nc.vector.tensor_tensor(
                        out=agg[:], in0=n1[:], in1=n2[:], op=Alu.add
                    )
                    d1 = opool.tile([CN, 1], f32, tag="d1")
                    nc.vector.tensor_scalar(
                        out=d1[:], in0=p1[:, HC : HC + 1], scalar1=B1[0:CN, :],
                        scalar2=None, op0=Alu.mult,
                    )
                    d2 = opool.tile([CN, 1], f32, tag="d2")
                    nc.vector.tensor_scalar(
                        out=d2[:], in0=p2[:, HC : HC + 1], scalar1=B2[0:CN, :],
                        scalar2=EPS, op0=Alu.mult, op1=Alu.add,
                    )
                    den = opool.tile([CN, 1], f32, tag="den")
                    nc.vector.tensor_tensor(
                        out=den[:], in0=d1[:], in1=d2[:], op=Alu.add
                    )
                    rec = opool.tile([CN, 1], f32, tag="rec")
                    nc.vector.reciprocal(out=rec[:], in_=den[:])
                    o1 = opool.tile([CN, HC], f32, tag="o1")
                    nc.vector.tensor_scalar(
                        out=o1[:], in0=agg[:], scalar1=rec[:], scalar2=None,
                        op0=Alu.mult,
                    )
                    o2 = opool.tile([CN, HC], f32, tag="o2")
                    nc.vector.tensor_tensor(
                        out=o2[:], in0=o1[:], in1=own[:, 0:HC], op=Alu.add
                    )
                    o3 = opool.tile([CN, HC], f32, tag="o3")
                    nc.vector.tensor_tensor(
                        out=o3[:], in0=o2[:], in1=biasb[0:CN, :], op=Alu.add
                    )
                    nc.sync.dma_start(
                        out=out_h.ap()[c * CN : (c + 1) * CN, :], in_=o3[:]
                    )

    nc.compile()
    return nc


def _get_program(cfg, TB):
    key = (tuple(sorted(cfg.items())), TB)
    if key not in _PROG_CACHE:
        _PROG_CACHE[key] = build_program(cfg, TB)
    return _PROG_CACHE[key]


# ---------------------------------------------------------------- entry
def run(cfg, inputs, _profile=None):
    _ensure_ntff_hook()
    from concourse.bass_utils import run_bass_kernel_spmd

    in_maps, TB = host_prep(cfg, **inputs)
    nc = _get_program(cfg, TB)
    kwargs = {}
    if _profile is not None:
        kwargs = dict(trace=True, tmpdir=_profile)
    res = run_bass_kernel_spmd(
        nc, in_maps, core_ids=list(range(cfg["NCORES"])), **kwargs
    )
    out = np.concatenate(
        [res.results[k]["out"] for k in range(cfg["NCORES"])], axis=0
    )[: cfg["N"]]
    run.last_exec_time_ns = res.exec_time_ns
    return np.ascontiguousarray(out, dtype=np.float32)


def kernel(x, edge_index, v_mapping, W_src, att_src, att_dst, bias, _profile=None):
    inputs = dict(
        x=np.asarray(x, np.float32),
        edge_index=np.asarray(edge_index),
        v_mapping=np.asarray(v_mapping, np.float32),
        W_src=np.asarray(W_src, np.float32),
        att_src=np.asarray(att_src, np.float32),
        att_dst=np.asarray(att_dst, np.float32),
        bias=np.asarray(bias, np.float32),
    )
    out = run(FULL_CFG, inputs, _profile=_profile)
    kernel.last_exec_time_ns = run.last_exec_time_ns
    return out


# revision 24
# speedup vs baseline: 1.1669x; 1.0239x over previous
"""GAT attention-locator message passing on 8 Trainium2 NeuronCores.

Strategy: shard by destination node (N/8 dst nodes per core) so the
segment softmax is fully core-local.

Key factorization: with the global softmax shift C (valid per segment by
shift invariance) and the continuity of leaky_relu at 0, each edge weight
is  w = exp(as+ad-C)            if as+ad > 0
     w = exp(0.2(as+ad)-C)      otherwise
Both branches factor into (src term)x(dst term):
     w = A1[src]*B1[dst]  or  A2[src]*B2[dst]
  A1 = exp(as-C/2), B1 = exp(ad-C/2), A2 = exp(0.2as-C/2), B2 = exp(0.2ad-C/2).
The host classifies edges by sign(as+ad) (computed on CPU for LAYOUT only;
misclassification at the boundary is harmless since both branches agree
at 0).  The device then needs NO per-edge scalars at all:
 - A-terms are folded into pre-scaled gather tables (built on device):
     xc row m (bf16, 384 slots): [xp*A1 (128) | A1 | xp*A2 (128) | A2 | pad]
 - B-terms are applied per NODE after PSUM accumulation.
Per edge: one 768B dma_gather row + a shared 0/1 mask + one bf16 matmul
accumulating messages and the denominator (A column) into PSUM.

Per chunk (125 dst nodes): 4 gathers (sign x int16-index-range split),
one tiny per-node gather for ad (B-terms), T matmuls, per-node combine:
  out = (B1*psum1 + B2*psum2) / (B1*den1 + B2*den2 + eps) + x_proj + bias
"""

import math
import sys
import types

import numpy as np

# ------------------------------------------------------------ configuration
def make_cfg(N, E, F, HC, NCORES, CN, SPLIT, CSHIFT=12.0):
    NPC = N // NCORES
    assert NPC % CN == 0
    NPAD = ((N + 1023) // 1024) * 1024  # divisible by 8*128
    OWNK = (NPC + 127) // 128  # rows per partition in core-local tables
    cfg = dict(
        N=N, E=E, F=F, HC=HC, NCORES=NCORES, NPC=NPC, CN=CN,
        CHUNKS=NPC // CN, GCHUNKS=NCORES * (NPC // CN), SPLIT=SPLIT,
        NPAD=NPAD, NTILES=NPAD // 128, NGRP=NPAD // 1024,
        OWNK=OWNK, NOWN=OWNK * 128,
        CSHIFT=CSHIFT, EPS=1e-16, NEG=0.2,
    )
    assert cfg["NOWN"] <= NPAD
    return cfg


FULL_CFG = make_cfg(N=50000, E=1600000, F=256, HC=128, NCORES=8, CN=125,
                    SPLIT=32768)

_PROG_CACHE: dict = {}


def _patch_ldw_opt():
    """Enable walrus LDWEIGHTS pipelining (merges weight loads into the
    matmul stream); the default pipeline disables it."""
    import concourse.bass_utils as bu

    if getattr(bu, "_ldw_patched", False):
        return
    orig = bu.run_command

    def patched(argv, **kw):
        argv = [a.replace("--enable-ldw-opt=false", "--enable-ldw-opt=true")
                if isinstance(a, str) else a for a in argv]
        return orig(argv, **kw)

    bu.run_command = patched
    bu._ldw_patched = True


def _ensure_ntff_hook():
    try:
        import antenv.axon_hooks  # noqa: F401

        return
    except ImportError:
        pass
    try:
        from trn_agent_boot.trn_boot import _ntff_profile_via_ctypes

        hook = _ntff_profile_via_ctypes("/opt/axon/libaxon_pjrt.so")
    except Exception:
        hook = None
    mod = types.ModuleType("antenv.axon_hooks")
    mod.get_axon_ntff_profile_hook = lambda: hook
    mod.set_axon_ntff_profile_hook = lambda h: None
    sys.modules["antenv.axon_hooks"] = mod


# ---------------------------------------------------------------- host prep
def _prep_edges(cfg, src, dst, possign):
    """4-way (sign, src-range) classed edge layout per dst chunk."""
    G, CN, SPLIT = cfg["GCHUNKS"], cfg["CN"], cfg["SPLIT"]
    src = src.astype(np.int64)
    dst = dst.astype(np.int64)
    g = dst // CN
    nl = dst % CN
    lowm = src < SPLIT
    # class: 0=pos-lo 1=pos-hi 2=neg-lo 3=neg-hi
    cls = np.where(possign, 0, 2) + np.where(lowm, 0, 1)

    key = (g * 4 + cls) * 65536 + src  # src-sorted within class: HBM locality
    order = np.argsort(key, kind="stable")
    key = key // 65536
    g_s, nl_s, src_s, cls_s = g[order], nl[order], src[order], cls[order]
    key_s = key[order]
    starts = np.searchsorted(key_s, np.arange(4 * G))
    counts = np.bincount(key_s, minlength=4 * G)
    TB = [max(1, math.ceil(counts[c::4].max() / 128)) for c in range(4)]
    T = sum(TB)
    off = np.cumsum([0] + TB[:-1]) * 128  # class position offsets

    rank = np.arange(len(order)) - starts[key_s]
    pos = off[cls_s] + rank
    p = (pos % 128).astype(np.int64)
    t = (pos // 128).astype(np.int64)

    dstl = np.full((G, 128, T), -1.0, np.float32)
    dstl[g_s, p, t] = nl_s.astype(np.float32)

    idx = []
    for c in range(4):
        m = cls_s == c
        a = np.zeros((G, TB[c] * 128), np.int16)
        v = src_s[m] - (SPLIT if c % 2 else 0)
        a[g_s[m], rank[m]] = v.astype(np.int16)
        idx.append(a)

    def wrap(a):
        tb8 = a.shape[1] // 16
        w = a.reshape(G, tb8, 16).transpose(0, 2, 1)
        return np.ascontiguousarray(np.tile(w, (1, 8, 1)))

    # per-(chunk, class) valid-index counts, maxed over cores (SPMD-uniform)
    CH = G // cfg["NCORES"]
    cnt = counts.reshape(G, 4).reshape(cfg["NCORES"], CH, 4)
    ni = cnt.max(axis=0)  # [CHUNKS, 4]
    ni = np.minimum(((ni + 127) // 128) * 128, np.array(TB) * 128)
    ni = np.maximum(ni, 128)
    return dict(TB=tuple(TB), T=T, dstl=dstl, ni=tuple(map(tuple, ni)),
                idx=[wrap(a) for a in idx])


def host_prep(cfg, x, edge_index, v_mapping, W_src, att_src, att_dst, bias):
    N, F = cfg["N"], cfg["F"]
    NCORES, NPC, CN, CHUNKS = cfg["NCORES"], cfg["NPC"], cfg["CN"], cfg["CHUNKS"]
    NPAD, OWNK = cfg["NPAD"], cfg["OWNK"]

    def _fold(att):
        a = att[0]
        a = a / max(np.linalg.norm(a), 1e-12)
        return v_mapping.T @ a

    u_src = _fold(att_src)
    u_dst = _fold(att_dst)
    w_ext = np.concatenate(
        [W_src, u_src[:, None], u_dst[:, None]], axis=1
    ).astype(np.float32)

    x_pad = np.zeros((NPAD, F), np.float32)
    x_pad[:N] = x

    # sign classification (layout only; boundary-safe because the two
    # leaky-relu branches agree at 0)
    a_src_h = x @ u_src.astype(np.float32)
    a_dst_h = x @ u_dst.astype(np.float32)
    src, dst = edge_index[0].astype(np.int64), edge_index[1].astype(np.int64)
    possign = (a_src_h[src] + a_dst_h[dst]) > 0

    tabs = _prep_edges(cfg, src, dst, possign)

    import ml_dtypes
    iota_n = np.tile(np.arange(CN, dtype=np.float32), (128, 1))
    bias_b = np.tile(bias[None, :], (128, 1)).astype(np.float32)
    ident = np.eye(128, dtype=np.float32)

    # per-chunk per-node small-gather indices (core-independent)
    nidx = np.zeros((CHUNKS, 128), np.int16)
    for c in range(CHUNKS):
        nidx[c] = c * CN + np.minimum(np.arange(128), CN - 1)
    flat = nidx.reshape(-1)  # [CHUNKS*128]
    nidx_w = np.ascontiguousarray(
        np.tile(flat.reshape(CHUNKS * 8, 16).T, (8, 1))
    )  # [128, CHUNKS*8]

    T = tabs["T"]
    iota_rep = np.ascontiguousarray(
        np.broadcast_to(np.arange(CN, dtype=np.float32), (128, T, CN))
    ).astype(ml_dtypes.bfloat16)
    p128 = np.arange(128, dtype=np.int32)
    in_maps = []
    for k in range(NCORES):
        sl = slice(k * CHUNKS, (k + 1) * CHUNKS)
        base = k * NPC
        in_maps.append(
            {
                "x_pad": x_pad,
                "w_ext": w_ext,
                "iota_n": iota_n,
                "bias_b": bias_b,
                "ident": ident,
                "own_off": (base + p128 * OWNK).astype(np.int32)[:, None],
                "gidx0": tabs["idx"][0][sl],
                "gidx1": tabs["idx"][1][sl],
                "gidx2": tabs["idx"][2][sl],
                "gidx3": tabs["idx"][3][sl],
                "nidx": nidx_w,
                "iota_rep": iota_rep,
                "dstl": tabs["dstl"][sl].astype(ml_dtypes.bfloat16),
            }
        )
    return in_maps, (tabs["TB"], tabs["ni"])


# ------------------------------------------------------------- bass program
def build_program(cfg, TB, NI):
    import concourse.mybir as mybir
    import concourse.tile as tile
    from concourse import bacc
    from concourse.bass import IndirectOffsetOnAxis

    F, HC = cfg["F"], cfg["HC"]
    NCORES, CN, CHUNKS = cfg["NCORES"], cfg["CN"], cfg["CHUNKS"]
    NPC, NPAD, NTILES, NGRP = cfg["NPC"], cfg["NPAD"], cfg["NTILES"], cfg["NGRP"]
    SPLIT, OWNK, NOWN = cfg["SPLIT"], cfg["OWNK"], cfg["NOWN"]
    CSHIFT, EPS, NEG = cfg["CSHIFT"], cfg["EPS"], cfg["NEG"]
    KB = F // 128
    XW = 2 * HC  # table row slots: [xp*A (128) | A | junk] in 256 (512B)
    C2 = CSHIFT / 2.0

    T = sum(TB)
    TPOS = TB[0] + TB[1]
    f32 = mybir.dt.float32
    bf16 = mybir.dt.bfloat16
    i16 = mybir.dt.int16
    Alu = mybir.AluOpType
    Act = mybir.ActivationFunctionType

    nc = bacc.Bacc("TRN2", target_bir_lowering=False, debug=False, num_devices=NCORES)

    x_h = nc.dram_tensor("x_pad", [NPAD, F], f32, kind="ExternalInput")
    w_h = nc.dram_tensor("w_ext", [F, HC + 2], f32, kind="ExternalInput")
    iota_h = nc.dram_tensor("iota_n", [128, CN], f32, kind="ExternalInput")
    biasb_h = nc.dram_tensor("bias_b", [128, HC], f32, kind="ExternalInput")
    ident_h = nc.dram_tensor("ident", [128, 128], f32, kind="ExternalInput")
    ownoff_h = nc.dram_tensor("own_off", [128, 1], mybir.dt.int32, kind="ExternalInput")
    gidx_h = [
        nc.dram_tensor(f"gidx{c}", [CHUNKS, 128, TB[c] * 8], i16,
                       kind="ExternalInput")
        for c in range(4)
    ]
    nidx_h = nc.dram_tensor("nidx", [128, CHUNKS * 8], i16, kind="ExternalInput")
    iotar_h = nc.dram_tensor("iota_rep", [128, T, CN], bf16, kind="ExternalInput")
    dstl_h = nc.dram_tensor("dstl", [CHUNKS, 128, T], bf16, kind="ExternalInput")
    out_h = nc.dram_tensor("out", [NPC, HC], f32, kind="ExternalOutput")

    with tile.TileContext(nc) as tc:
        with tc.tile_pool(name="dram", bufs=1, space="DRAM") as dpool:
            xp_t = dpool.tile([NPAD, XW], bf16)     # pos-scaled rows
            xn_t = dpool.tile([NPAD, XW], bf16)     # neg-scaled rows
            xr_d = dpool.tile([NPAD, HC], bf16)     # raw x_proj rows
            scT_d = dpool.tile([NPAD, 1], f32)      # a_dst node-contiguous
            sc3_d = dpool.tile([NOWN, HC], bf16)    # [ad_hi|ad_lo|junk] rows
            ownxp_d = dpool.tile([NOWN, HC], bf16)  # own raw x_proj rows

            # ---------------- Phase P: projection + tables ----------------
            with (
                tc.tile_pool(name="pconst", bufs=1) as cpool,
                tc.tile_pool(name="pio", bufs=3) as iopool,
                tc.tile_pool(name="pwork", bufs=4) as wpool,
                tc.tile_pool(name="ppsum", bufs=2, space="PSUM") as pp,
            ):
                wk = []
                for kb in range(KB):
                    wt = cpool.tile([128, HC + 2], f32, tag=f"w{kb}")
                    nc.sync.dma_start(
                        out=wt[:], in_=w_h.ap()[kb * 128 : (kb + 1) * 128, :]
                    )
                    wk.append(wt)
                ident = cpool.tile([128, 128], f32)
                nc.sync.dma_start(out=ident[:], in_=ident_h.ap())
                nC2p = cpool.tile([128, 1], f32)
                nc.vector.memset(nC2p[:], -C2)
                adst_sbuf = cpool.tile([128, NTILES], f32)

                x_r = x_h.ap().rearrange("(g a p) f -> g p a f", a=8, p=128)
                xp_r = xp_t[:].rearrange("(g a p) c -> g p a c", a=8, p=128)
                xn_r = xn_t[:].rearrange("(g a p) c -> g p a c", a=8, p=128)
                xr_r = xr_d[:].rearrange("(g a p) c -> g p a c", a=8, p=128)
                for gi in range(NGRP):
                    xt = iopool.tile([128, 8, F], f32)
                    nc.sync.dma_start(out=xt[:], in_=x_r[gi])
                    xep = iopool.tile([128, 8, XW], bf16, tag="xep")
                    nc.gpsimd.memset(xep[:, :, HC + 1 : XW], 0.0)
                    xen = iopool.tile([128, 8, XW], bf16, tag="xen")
                    nc.gpsimd.memset(xen[:, :, HC + 1 : XW], 0.0)
                    xer = iopool.tile([128, 8, HC], bf16, tag="xer")
                    for a in range(8):
                        nt = gi * 8 + a
                        px = pp.tile([128, HC + 2], f32, tag="px")
                        for kb in range(KB):
                            pt = pp.tile([128, 128], f32, tag=f"pt{kb}")
                            nc.tensor.transpose(
                                out=pt[:],
                                in_=xt[:, a, kb * 128 : (kb + 1) * 128],
                                identity=ident[:],
                            )
                            xT = wpool.tile([128, 128], f32, tag=f"xT{kb}")
                            nc.vector.tensor_copy(out=xT[:], in_=pt[:])
                            nc.tensor.matmul(
                                px[:],
                                lhsT=xT[:],
                                rhs=wk[kb][:],
                                start=(kb == 0),
                                stop=(kb == KB - 1),
                            )
                        A1f = wpool.tile([128, 1], f32, tag="A1f")
                        nc.scalar.activation(
                            out=A1f[:], in_=px[:, HC : HC + 1], func=Act.Exp,
                            bias=nC2p[:], scale=1.0,
                        )
                        A2f = wpool.tile([128, 1], f32, tag="A2f")
                        nc.scalar.activation(
                            out=A2f[:], in_=px[:, HC : HC + 1], func=Act.Exp,
                            bias=nC2p[:], scale=NEG,
                        )
                        nc.vector.tensor_scalar(
                            out=xep[:, a, 0:HC], in0=px[:, 0:HC], scalar1=A1f[:],
                            scalar2=None, op0=Alu.mult,
                        )
                        nc.gpsimd.tensor_copy(
                            out=xep[:, a, HC : HC + 1], in_=A1f[:]
                        )
                        nc.vector.tensor_scalar(
                            out=xen[:, a, 0:HC], in0=px[:, 0:HC],
                            scalar1=A2f[:], scalar2=None, op0=Alu.mult,
                        )
                        nc.gpsimd.tensor_copy(
                            out=xen[:, a, HC : HC + 1], in_=A2f[:]
                        )
                        nc.vector.tensor_copy(out=xer[:, a, :], in_=px[:, 0:HC])
                        nc.vector.tensor_copy(
                            out=adst_sbuf[:, nt : nt + 1],
                            in_=px[:, HC + 1 : HC + 2],
                        )
                    nc.sync.dma_start(out=xp_r[gi], in_=xep[:])
                    nc.sync.dma_start(out=xn_r[gi], in_=xen[:])
                    nc.sync.dma_start(out=xr_r[gi], in_=xer[:])
                scT_r = scT_d[:].rearrange("(nt p) one -> p (nt one)", p=128)
                nc.sync.dma_start(out=scT_r, in_=adst_sbuf[:])

                ownoff = cpool.tile([128, 1], mybir.dt.int32)
                nc.sync.dma_start(out=ownoff[:], in_=ownoff_h.ap())
                own_ad = cpool.tile([128, OWNK], f32)
                nc.gpsimd.indirect_dma_start(
                    out=own_ad[:], out_offset=None, in_=scT_d[:],
                    in_offset=IndirectOffsetOnAxis(ap=ownoff[:], axis=0),
                )
                sc3s = cpool.tile([128, OWNK, HC], bf16)
                nc.gpsimd.memset(sc3s[:], 0.0)
                nc.vector.tensor_copy(out=sc3s[:, :, 0], in_=own_ad[:])
                nc.vector.tensor_tensor(
                    out=sc3s[:, :, 1], in0=own_ad[:], in1=sc3s[:, :, 0],
                    op=Alu.subtract,
                )
                sc3_r = sc3_d[:].rearrange("(p i) c -> p i c", p=128)
                nc.sync.dma_start(out=sc3_r, in_=sc3s[:])
                own_xe = cpool.tile([128, OWNK * HC], bf16)
                nc.gpsimd.indirect_dma_start(
                    out=own_xe[:], out_offset=None, in_=xr_d[:],
                    in_offset=IndirectOffsetOnAxis(ap=ownoff[:], axis=0),
                )
                oxp_r = ownxp_d[:].rearrange("(p i) c -> p (i c)", p=128)
                nc.sync.dma_start(out=oxp_r, in_=own_xe[:])

            # ---------------- Phase E: edges ------------------------------
            with (
                tc.tile_pool(name="econst", bufs=1) as ecpool,
                tc.tile_pool(name="eidx", bufs=2) as xpool,
                tc.tile_pool(name="egath", bufs=3) as gpool,
                tc.tile_pool(name="ework", bufs=4) as epool,
                tc.tile_pool(name="esel", bufs=4) as spool,
                tc.tile_pool(name="eout", bufs=2) as opool,
                tc.tile_pool(name="epsum", bufs=3, space="PSUM") as ep,
            ):
                iota_r = ecpool.tile([128, T, CN], bf16)
                nc.sync.dma_start(out=iota_r[:], in_=iotar_h.ap())
                nixall = ecpool.tile([128, CHUNKS * 8], i16)
                nc.sync.dma_start(out=nixall[:], in_=nidx_h.ap())
                ADall = ecpool.tile([128, CHUNKS, HC], bf16)
                nc.gpsimd.dma_gather(
                    out_ap=ADall[:], in_ap=sc3_d[:], idxs_ap=nixall[:],
                    num_idxs=CHUNKS * 128, num_idxs_reg=CHUNKS * 128,
                    elem_size=HC, single_packet=False,
                )
                biasb = ecpool.tile([128, HC], f32)
                nc.sync.dma_start(out=biasb[:], in_=biasb_h.ap())
                nC2e = ecpool.tile([128, 1], f32)
                nc.vector.memset(nC2e[:], -C2)

                for c in range(CHUNKS):
                    gidx_t = []
                    for cl in range(4):
                        gt = xpool.tile([128, TB[cl] * 8], i16, tag=f"g{cl}")
                        nc.sync.dma_start(out=gt[:], in_=gidx_h[cl].ap()[c])
                        gidx_t.append(gt)
                    dl = xpool.tile([128, T], bf16, tag="dl")
                    nc.sync.dma_start(out=dl[:], in_=dstl_h.ap()[c])
                    own = xpool.tile([CN, HC], bf16, tag="own")
                    nc.sync.dma_start(
                        out=own[:], in_=ownxp_d[:][c * CN : (c + 1) * CN, :]
                    )

                    G = gpool.tile([128, T, XW], bf16)
                    toff = 0
                    for cl in range(4):
                        lo, hi = (0, SPLIT) if cl % 2 == 0 else (SPLIT, NPAD)
                        tab = xp_t if cl < 2 else xn_t
                        ni = NI[c][cl]
                        nt_ = ni // 128
                        nc.gpsimd.dma_gather(
                            out_ap=G[:, toff : toff + nt_, :],
                            in_ap=tab[:][lo:hi, :],
                            idxs_ap=gidx_t[cl][:, 0 : ni // 16],
                            num_idxs=ni,
                            num_idxs_reg=ni,
                            elem_size=XW,
                            single_packet=False,
                        )
                        if nt_ < TB[cl]:  # stale tiles: keep finite (0*NaN=NaN)
                            nc.gpsimd.memset(
                                G[:, toff + nt_ : toff + TB[cl], :], 0.0
                            )
                        toff += TB[cl]
                    adf = epool.tile([128, 1], f32, tag="adf")
                    nc.vector.tensor_tensor(
                        out=adf[:], in0=ADall[:, c, 0:1], in1=ADall[:, c, 1:2],
                        op=Alu.add,
                    )
                    B1 = epool.tile([128, 1], f32, tag="B1")
                    nc.scalar.activation(out=B1[:], in_=adf[:], func=Act.Exp,
                                         bias=nC2e[:], scale=1.0)
                    B2 = epool.tile([128, 1], f32, tag="B2")
                    nc.scalar.activation(out=B2[:], in_=adf[:], func=Act.Exp,
                                         bias=nC2e[:], scale=NEG)

                    S0a = spool.tile([128, T, CN], bf16, tag="S0a")
                    dl3 = dl[:].rearrange("p (t one) -> p t one", one=1)
                    nc.vector.tensor_tensor(
                        out=S0a[:], in0=dl3.to_broadcast([128, T, CN]),
                        in1=iota_r[:], op=Alu.is_equal,
                    )
                    p1 = ep.tile([CN, HC + 1], f32, tag="p1")
                    p2 = ep.tile([CN, HC + 1], f32, tag="p2")
                    for t in range(T):
                        pos = t < TPOS
                        ps = p1 if pos else p2
                        nc.tensor.matmul(
                            ps[:],
                            lhsT=S0a[:, t, :],
                            rhs=G[:, t, 0 : HC + 1],
                            start=(t == 0 or t == TPOS),
                            stop=(t == TPOS - 1 or t == T - 1),
                        )

                    n1 = opool.tile([CN, HC], f32, tag="n1")
                    nc.vector.tensor_scalar(
                        out=n1[:], in0=p1[:, 0:HC], scalar1=B1[0:CN, :],
                        scalar2=None, op0=Alu.mult,
                    )
                    n2 = opool.tile([CN, HC], f32, tag="n2")
                    nc.vector.tensor_scalar(
                        out=n2[:], in0=p2[:, 0:HC], scalar1=B2[0:CN, :],
                        scalar2=None, op0=Alu.mult,
                    )
                    agg = opool.tile([CN, HC], f32, tag="agg")
                    nc.vector.tensor_tensor(
                        out=agg[:], in0=n1[:], in1=n2[:], op=Alu.add
                    )
                    d1 = opool.tile([CN, 1], f32, tag="d1")
                    nc.vector.tensor_scalar(
                        out=d1[:], in0=p1[:, HC : HC + 1], scalar1=B1[0:CN, :],
                        scalar2=None, op0=Alu.mult,
                    )
                    d2 = opool.tile([CN, 1], f32, tag="d2")
                    nc.vector.tensor_scalar(
                        out=d2[:], in0=p2[:, HC : HC + 1], scalar1=B2[0:CN, :],
                        scalar2=EPS, op0=Alu.mult, op1=Alu.add,
                    )
                    den = opool.tile([CN, 1], f32, tag="den")
                    nc.vector.tensor_tensor(
                        out=den[:], in0=d1[:], in1=d2[:], op=Alu.add
                    )
                    rec = opool.tile([CN, 1], f32, tag="rec")
                    nc.vector.reciprocal(out=rec[:], in_=den[:])
                    o1 = opool.tile([CN, HC], f32, tag="o1")
                    nc.vector.tensor_scalar(
                        out=o1[:], in0=agg[:], scalar1=rec[:], scalar2=None,
                        op0=Alu.mult,
                    )
                    o2 = opool.tile([CN, HC], f32, tag="o2")
                    nc.vector.tensor_tensor(
                        out=o2[:], in0=o1[:], in1=own[:, 0:HC], op=Alu.add
                    )
                    o3 = opool.tile([CN, HC], f32, tag="o3")
                    nc.vector.tensor_tensor(
                        out=o3[:], in0=o2[:], in1=biasb[0:CN, :], op=Alu.add
                    )
                    nc.sync.dma_start(
                        out=out_h.ap()[c * CN : (c + 1) * CN, :], in_=o3[:]
                    )

    nc.compile()
    return nc


def _get_program(cfg, TB, NI):
    key = (tuple(sorted(cfg.items())), TB, NI)
    if key not in _PROG_CACHE:
        _PROG_CACHE[key] = build_program(cfg, TB, NI)
    return _PROG_CACHE[key]


# ---------------------------------------------------------------- entry
def run(cfg, inputs, _profile=None):
    _ensure_ntff_hook()
    from concourse.bass_utils import run_bass_kernel_spmd

    in_maps, (TB, NI) = host_prep(cfg, **inputs)
    nc = _get_program(cfg, TB, NI)
    kwargs = {}
    if _profile is not None:
        kwargs = dict(trace=True, tmpdir=_profile)
    res = run_bass_kernel_spmd(
        nc, in_maps, core_ids=list(range(cfg["NCORES"])), **kwargs
    )
    out = np.concatenate(
        [res.results[k]["out"] for k in range(cfg["NCORES"])], axis=0
    )[: cfg["N"]]
    run.last_exec_time_ns = res.exec_time_ns
    return np.ascontiguousarray(out, dtype=np.float32)


def kernel(x, edge_index, v_mapping, W_src, att_src, att_dst, bias, _profile=None):
    inputs = dict(
        x=np.asarray(x, np.float32),
        edge_index=np.asarray(edge_index),
        v_mapping=np.asarray(v_mapping, np.float32),
        W_src=np.asarray(W_src, np.float32),
        att_src=np.asarray(att_src, np.float32),
        att_dst=np.asarray(att_dst, np.float32),
        bias=np.asarray(bias, np.float32),
    )
    out = run(FULL_CFG, inputs, _profile=_profile)
    kernel.last_exec_time_ns = run.last_exec_time_ns
    return out
